# revision 2
# baseline (speedup 1.0000x reference)
"""MoIE (mixture of implicit experts) Trainium2 kernel.

Math (per reference):
    alpha = softmax(x @ gate_W + gate_b)                    # (B, K)
    h = x
    for l in 0..3:  h = relu(sum_k alpha_k * (h @ W[l,k] + b[l,k]))
    out = sum_k alpha_k * (h @ out_W[k] + out_b[k])

Strategy (v2 — o2-major sweeps, split PSUM pools):
  - Data-parallel: shard B=32768 tokens over 8 cores (4096 each); replicate
    the small weights. No collectives.
  - Feature-major on device: activations live as hT [D(part), T(free)] so
    chained matmuls need no activation transposes.
  - alpha folded into the *moving* operand: rhs_k = hT * bcast(alphaT[k]);
    PSUM accumulates over experts and contraction chunks; the per-expert
    bias enters as a tiny alphaT-contraction matmul; the gate bias enters
    as the per-partition bias of the exp() activation.
  - T=1024 tokens per tile: a z accumulator [128,1024]f32 is 2 PSUM banks,
    so zpool holds 3 buffers (6 banks) and the gate gets its own 2-bank
    pool — the pipelined gate no longer serializes the next layer's PSUM
    allocation (the 427us build shared one 2-buffer pool for both).
  - o2-major layer sweeps: all (k,i2) expert matmuls for output-half 0
    accumulate + evacuate BEFORE the o2=1 sweep runs. z[o2=0] therefore
    finishes mid-layer and its relu/evac + the next layer's DVE rhs
    production hide under ~7us of o2=1 matmuls (the 427us build finished
    both halves at the layer end, exposing the evac->rhs chain).
  - rhs tiles live across both o2 sweeps: rpool bufs=24 (2KB/partition
    each) so the DVE runs a full layer ahead.
  - fp16 matmul path, fp32 PSUM; fp16 output bounce.
  - LDWEIGHTS dedupe within same-stationary groups (PE stream pinned to
    program order); measured 1055/1062 LDW hidden under matmuls.
  - Software-pipelined gating: tile ti's layer pass carries tile ti+1's
    gate (cyclic across reps at the last tile); with NT=4 the next-rep
    broadcast lands ~3 layer-tiles before the For_i barrier.
"""

import sys

if "/opt/trn_rl_repo" not in sys.path:
    sys.path.insert(0, "/opt/trn_rl_repo")

import numpy as np

import concourse.bass as bass
import concourse.bass_isa as bass_isa
import concourse.tile as tile
import concourse.mybir as mybir
from concourse import bacc
from concourse.bass import _add_dep_helper
from concourse.bass_utils import run_bass_kernel_spmd

N_CORES = 8
B, D, K, L = 32768, 256, 8, 4
NL = L + 1                  # 4 hidden blocks + output block
BS = B // N_CORES           # 4096 tokens per core
T = 1024                    # tokens per on-chip tile
NT = BS // T                # tiles per core
SEG = 512                   # f32 PSUM bank = 512 elements
NSEG = T // SEG
F16 = mybir.dt.float16
F32 = mybir.dt.float32
AF = mybir.ActivationFunctionType
_RPOOL_BUFS = 24
_APOOL_BUFS = 2
_ZPOOL_BUFS = 3
_GPOOL_BUFS = 2
_LDW_DEDUP = True


class _MMEmitter:
    """Emit matmuls, tracking which ones share a stationary operand with the
    immediately preceding matmul. Tile's legalizer splits every InstMatmult
    into InstLdweights + InstMatmult; `_dedupe_ldweights` later deletes the
    redundant loads for the marked matmuls. A nosync dep chain pins the PE
    stream to program order so a dedup'd matmul can never observe a foreign
    group's weights."""

    def __init__(self, nc):
        self.nc = nc
        self.key = None
        self.prev = None
        self.skip_names = set()

    def mm(self, key, out, lhsT, rhs, start, stop):
        bi = self.nc.tensor.matmul(out, lhsT, rhs, start=start, stop=stop)
        if _LDW_DEDUP:
            if self.prev is not None:
                _add_dep_helper(
                    bi.ins, self.prev, sync=False, reason="pe-program-order"
                )
            if key is not None and key == self.key:
                self.skip_names.add(bi.ins.name)
        self.key = key
        self.prev = bi.ins
        return bi


def _dedupe_ldweights(nc, skip_names):
    """Remove the InstLdweights preceding each marked matmul (same stationary
    as the previous matmul, PE stream pinned to program order). Waits are
    moved onto the matmul; dependency edges are merged / remapped."""
    removed = {}
    for b in nc.m.functions[0].blocks:
        insts = list(b.instructions)
        keep = [True] * len(insts)
        for idx, ins in enumerate(insts):
            if not (isinstance(ins, mybir.InstMatmult) and ins.name in skip_names):
                continue
            j = idx - 1
            lw = None
            while j >= 0:
                pj = insts[j]
                if isinstance(pj, mybir.InstLdweights):
                    if keep[j]:
                        lw = pj
                    break
                if isinstance(pj, mybir.InstMatmult):
                    break
                j -= 1
            if lw is None:
                continue
            si = lw.sync_info
            if si is not None and len(si.on_update) > 0:
                continue  # LDW signals a semaphore: leave it alone
            if si is not None and len(si.on_wait) > 0:
                msi = ins.sync_info
                waits = list(si.on_wait) + (
                    list(msi.on_wait) if msi is not None else []
                )
                upds = list(msi.on_update) if msi is not None else []
                ins.sync_info = mybir.SyncInfo(on_wait=waits, on_update=upds)
            ins.merge_dependencies_from(lw)
            keep[j] = False
            removed[lw.name] = ins.name
        if not all(keep):
            b.instructions = [i for i, k in zip(insts, keep) if k]
    if removed:
        for b in nc.m.functions[0].blocks:
            for i in b.instructions:
                i.remap_dependency_names(removed)
    return len(removed)


def _build_kernel(reps=1):
    nc = bacc.Bacc(
        "TRN2",
        target_bir_lowering=False,
        debug=False,
        enable_asserts=False,
        num_devices=N_CORES,
    )
    xT = nc.dram_tensor("xT", [D, BS], F16, kind="ExternalInput").ap()
    w = nc.dram_tensor("w", [NL, K, D, D], F16, kind="ExternalInput").ap()
    bb = nc.dram_tensor("bb", [NL, K, D], F16, kind="ExternalInput").ap()
    gw = nc.dram_tensor("gw", [D, K], F16, kind="ExternalInput").ap()
    gbc = nc.dram_tensor("gbc", [K, 1], F16, kind="ExternalInput").ap()
    outT = nc.dram_tensor("outT", [D, BS], F16, kind="ExternalOutput").ap()

    with tile.TileContext(nc) as tc:
        em = _body(nc, tc, xT, w, bb, gw, gbc, outT, reps)
    if _LDW_DEDUP and em is not None:
        n = _dedupe_ldweights(nc, em.skip_names)
        assert n == len(em.skip_names), (n, len(em.skip_names))
    nc.compile()
    return nc


def _body(nc, tc, xT, w, bb, gw, gbc_d, outT, reps=1):
    with (
        tc.tile_pool(name="cpool", bufs=1) as cpool,
        tc.tile_pool(name="xpool", bufs=2 * NT) as xpool,
        tc.tile_pool(name="hpool", bufs=6) as hpool,
        tc.tile_pool(name="rpool", bufs=_RPOOL_BUFS) as rpool,
        tc.tile_pool(name="apool", bufs=_APOOL_BUFS) as apool,
        tc.tile_pool(name="spool", bufs=2) as spool,
        tc.tile_pool(name="opool", bufs=2) as opool,
        tc.tile_pool(name="dpool", bufs=2, space=bass.MemorySpace.DRAM) as dpool,
        tc.tile_pool(name="zpool", bufs=_ZPOOL_BUFS, space=bass.MemorySpace.PSUM) as zpool,
        tc.tile_pool(name="gpool", bufs=_GPOOL_BUFS, space=bass.MemorySpace.PSUM) as gpool,
    ):
        # ---- small constants first (the HWDGE queue is FIFO: keep the
        # gate/bias/x transfers ahead of the 5MB weight stream) ----
        gwt = cpool.tile([128, 2 * K], F16, name="gwt")
        for i2 in range(2):
            nc.sync.dma_start(
                gwt[:, i2 * K : (i2 + 1) * K], gw[i2 * 128 : (i2 + 1) * 128, :]
            )
        gbc = cpool.tile([K, 1], F16, name="gbc")
        nc.sync.dma_start(gbc[:], gbc_d[:])
        bt = cpool.tile([K, NL * D], F16, name="bt")
        ones8x8 = cpool.tile([K, K], F16, name="ones8x8")
        nc.vector.memset(ones8x8[:], 1.0)
        wt = cpool.tile([128, NL * K * 2 * D], F16, name="wt")

        def load_weights():
            for l in range(NL):
                nc.sync.dma_start(bt[:, l * D : (l + 1) * D], bb[l])
            for l in range(NL):
                for k in range(K):
                    for i2 in range(2):
                        off = ((l * K + k) * 2 + i2) * D
                        nc.sync.dma_start(
                            wt[:, off : off + D],
                            w[l, k, i2 * 128 : (i2 + 1) * 128, :],
                        )

        def wslice(l, k, i2, o2):
            base = ((l * K + k) * 2 + i2) * D + o2 * 128
            return wt[:, base : base + 128]

        em = _MMEmitter(nc)

        if reps > 1:
            # steady-state benchmarking variant: weights resident across reps
            load_weights()

        hs, alphaTs, abcs = {}, {}, {}
        emit_seq = [0]

        def load_x(ti):
            sq = emit_seq[0]
            t0 = ti * T
            h = []
            for i2 in range(2):
                ht = xpool.tile([128, T], F16, tag="x", name=f"x_{ti}_{i2}_{sq}")
                for s in range(NSEG):
                    sl = slice(s * SEG, (s + 1) * SEG)
                    nc.sync.dma_start(
                        ht[:, sl],
                        xT[i2 * 128 : (i2 + 1) * 128, t0 + s * SEG : t0 + (s + 1) * SEG],
                    )
                h.append(ht)
            hs[ti] = h

        def gate_stage(ti):
            sq = emit_seq[0]
            emit_seq[0] += 1
            h = hs[ti]
            # ---- gate logits glT[k, t] = gate_W.T @ x; softmax over the 8
            # partitions. Gate PSUM comes from its own 2-bank pool so it never
            # blocks the layers' z allocation. ----
            eT = spool.tile([K, T], F16, tag="eT", name=f"eT_{ti}_{sq}", bufs=1)
            r8 = spool.tile([K, T], F16, tag="rT", name=f"r8_{ti}_{sq}", bufs=1)
            gate_stats = [
                (gwt[:, 0:K], h[0]),
                (gwt[:, K : 2 * K], h[1]),
            ]
            nstat = len(gate_stats)
            HT = T // 2
            for hf in range(2):
                glT = gpool.tile([K, HT], F32, tag="g", name=f"glT_{ti}_{hf}_{sq}")
                for wi, (stat, mov) in enumerate(gate_stats):
                    for s in range(NSEG // 2):
                        sl = slice(s * SEG, (s + 1) * SEG)
                        gsl = slice(hf * HT + s * SEG, hf * HT + (s + 1) * SEG)
                        em.mm(
                            ("gate", wi),
                            glT[:, sl],
                            stat,
                            mov[:, gsl],
                            start=(wi == 0),
                            stop=(wi == nstat - 1),
                        )
                # softmax over the 8 partitions (no max-subtract needed;
                # logits are ~N(0,1) so exp() is safe); gate bias enters as
                # the ACT per-partition bias vector
                hsl = slice(hf * HT, (hf + 1) * HT)
                nc.scalar.activation(eT[:, hsl], glT[:], AF.Exp, bias=gbc[:])
                # sum over experts broadcast back to all 8 partitions in one
                # go: all-ones [8,8] lhsT -> every row is sum_k e_k
                sT8 = gpool.tile([K, HT], F32, tag="g", name=f"sT8_{ti}_{hf}_{sq}")
                for s in range(NSEG // 2):
                    sl = slice(s * SEG, (s + 1) * SEG)
                    esl = slice(hf * HT + s * SEG, hf * HT + (s + 1) * SEG)
                    em.mm(
                        ("ones8",),
                        sT8[:, sl],
                        ones8x8[:],
                        eT[:, esl],
                        start=True,
                        stop=True,
                    )
                # per-hf reciprocal so sT8's PSUM bank frees before the next
                # hf needs it (gpool bufs=2)
                with nc.allow_low_precision("fp16 softmax normalizer"):
                    nc.vector.reciprocal(r8[:, hsl], sT8[:])
            alphaT = spool.tile([K, T], F16, tag="alphaT", name=f"alphaT_{ti}_{sq}")
            nc.vector.tensor_mul(alphaT[:], eT[:], r8[:])

            # broadcast alphaT rows across all 128 partitions so the DVE can
            # multiply h by alpha_k elementwise: bounce through DRAM (SBUF-
            # source broadcast APs are unsupported), then step-0 DRAM->SBUF
            # broadcast DMAs spread over 3 hardware queues
            abc = apool.tile([128, K * T], F16, tag="abc", name=f"abc_{ti}_{sq}")
            adram = dpool.tile([K, T], F16, tag="adram", name=f"adram_{ti}_{sq}")
            nc.sync.dma_start(adram[:], alphaT[:])
            bengs = [nc.sync, nc.scalar, nc.gpsimd]
            for k in range(K):
                bengs[k % len(bengs)].dma_start(
                    abc[:, k * T : (k + 1) * T],
                    adram[k : k + 1, :].broadcast_to([128, T]),
                )
            alphaTs[ti] = alphaT
            abcs[ti] = abc

            if ti == 0 and reps == 1:
                load_weights()

        def layers_stage(ti, after_l0=None):
            t0 = ti * T
            h = hs[ti]
            alphaT = alphaTs[ti]
            abc = abcs[ti]
            for l in range(NL):
                # rhs production in consumption order; tiles live across both
                # o2 sweeps (each is read by 2*NSEG matmuls per sweep)
                rhs = {}
                for k in range(K):
                    for i2 in range(2):
                        rt = rpool.tile(
                            [128, T], F16, tag="rhs", name=f"rhs_{ti}_{l}_{k}_{i2}"
                        )
                        nc.vector.tensor_mul(
                            rt[:], h[i2][:], abc[:, k * T : (k + 1) * T]
                        )
                        rhs[k, i2] = rt

                newh = []
                for o2 in range(2):
                    zt = zpool.tile([128, T], F32, tag="z", name=f"z_{ti}_{l}_{o2}")
                    # bias first (starts the accumulation group), then all
                    # experts; stop on the last expert per seg
                    for s in range(NSEG):
                        sl = slice(s * SEG, (s + 1) * SEG)
                        em.mm(
                            ("bt", l, o2),
                            zt[:, sl],
                            bt[:, l * D + o2 * 128 : l * D + (o2 + 1) * 128],
                            alphaT[:, sl],
                            start=True,
                            stop=False,
                        )
                    for k in range(K):
                        for i2 in range(2):
                            last = (k == K - 1) and (i2 == 1)
                            for s in range(NSEG):
                                sl = slice(s * SEG, (s + 1) * SEG)
                                em.mm(
                                    ("w", l, k, i2, o2),
                                    zt[:, sl],
                                    wslice(l, k, i2, o2),
                                    rhs[k, i2][:, sl],
                                    start=False,
                                    stop=last,
                                )
                    # immediate evacuation: o2=0's relu runs under the o2=1
                    # sweep, so the next layer's rhs production starts with
                    # ~7us of slack
                    if l < NL - 1:
                        nh = hpool.tile([128, T], F16, tag="h", name=f"h_{ti}_{l}_{o2}")
                        nc.scalar.activation(nh[:], zt[:], AF.Relu)
                        newh.append(nh)
                    else:
                        ot = opool.tile([128, T], F16, tag="o", name=f"out_{ti}_{o2}")
                        nc.scalar.activation(ot[:], zt[:], AF.Copy)
                        nc.sync.dma_start(
                            outT[o2 * 128 : (o2 + 1) * 128, t0 : t0 + T], ot[:]
                        )
                h = newh
                if l == 0 and after_l0 is not None:
                    # next tile's gate chain slots into this tile's L0->L1 PE
                    # boundary; its softmax/broadcast latency hides under
                    # layers 1..4
                    after_l0()

        # prologue: tile 0's gate outside the reps loop; each tile's layers
        # then carry the NEXT tile's gate (cyclic across reps)
        load_x(0)
        gate_stage(0)
        ctx = None
        if reps > 1:
            ctx = tc.For_i(0, reps, 1)
            ctx.__enter__()
        for ti in range(NT):
            nxt = ti + 1 if ti + 1 < NT else (0 if reps > 1 else None)
            cb = None
            if nxt is not None:
                def cb(n=nxt):
                    load_x(n)
                    gate_stage(n)
            layers_stage(ti, after_l0=cb)

        if ctx is not None:
            ctx.__exit__(None, None, None)
        return em


_NC_CACHE = None


def _get_nc():
    global _NC_CACHE
    if _NC_CACHE is None:
        _NC_CACHE = _build_kernel()
    return _NC_CACHE


class _Runner:
    """Persistent sharded PJRT executable for the bass kernel (compile once,
    run many). Mirrors bass2jax.run_bass_via_pjrt's multi-core branch minus
    buffer donation (the kernel writes every output element)."""

    def __init__(self, nc=None):
        import jax
        from jax.sharding import Mesh, PartitionSpec, NamedSharding
        from jax.experimental.shard_map import shard_map
        from concourse import bass2jax, mybir as _mybir

        self.jax = jax
        if nc is None:
            nc = _get_nc()
        bass2jax.install_neuronx_cc_hook()
        part_name = nc.partition_id_tensor.name if nc.partition_id_tensor else None
        in_names, out_names, out_avals, zero_outs = [], [], [], []
        for alloc in nc.m.functions[0].allocations:
            if not isinstance(alloc, _mybir.MemoryLocationSet):
                continue
            name = alloc.memorylocations[0].name
            if alloc.kind == "ExternalInput":
                if name != part_name:
                    in_names.append(name)
            elif alloc.kind == "ExternalOutput":
                out_names.append(name)
                shape = tuple(alloc.tensor_shape)
                dtype = _mybir.dt.np(alloc.dtype)
                out_avals.append(jax.core.ShapedArray(shape, dtype))
                zero_outs.append(np.zeros(shape, dtype))
        self.in_names, self.out_names, self.out_avals = in_names, out_names, out_avals

        bind_names = in_names + out_names + ([part_name] if part_name else [])

        def _body(*args):
            operands = list(args)
            if part_name is not None:
                operands.append(bass2jax.partition_id_tensor())
            outs = bass2jax._bass_exec_p.bind(
                *operands,
                out_avals=tuple(out_avals),
                in_names=tuple(bind_names),
                out_names=tuple(out_names),
                lowering_input_output_aliases=(),
                sim_require_finite=True,
                sim_require_nnan=True,
                nc=nc,
            )
            return tuple(outs)

        devices = jax.devices()[:N_CORES]
        self.mesh = Mesh(np.asarray(devices), ("core",))
        self.spec = PartitionSpec("core")
        self.sharding = NamedSharding(self.mesh, self.spec)
        n_args = len(in_names) + len(out_names)
        self.fn = jax.jit(
            shard_map(
                _body,
                mesh=self.mesh,
                in_specs=(self.spec,) * n_args,
                out_specs=(self.spec,) * len(out_names),
                check_rep=False,
            ),
            keep_unused=True,
        )
        self.zero_outs = [
            jax.device_put(
                np.zeros((N_CORES * z.shape[0], *z.shape[1:]), z.dtype), self.sharding
            )
            for z in zero_outs
        ]

    def device_inputs(self, in_maps):
        concat = [
            np.concatenate([np.asarray(m[name]) for m in in_maps], axis=0)
            for name in self.in_names
        ]
        return [self.jax.device_put(a, self.sharding) for a in concat]

    def run(self, dev_in):
        outs = self.fn(*dev_in, *self.zero_outs)
        return outs

    def to_maps(self, outs):
        res = []
        for c in range(N_CORES):
            res.append(
                {
                    name: np.asarray(outs[i]).reshape(
                        N_CORES, *self.out_avals[i].shape
                    )[c]
                    for i, name in enumerate(self.out_names)
                }
            )
        return res


_RUNNER = None


def _get_runner():
    global _RUNNER
    if _RUNNER is None:
        _RUNNER = _Runner()
    return _RUNNER


def _make_in_maps(x, gate_W, gate_b, block_W, block_b, out_W, out_b):
    x = np.asarray(x, dtype=np.float32)
    xT = np.ascontiguousarray(x.T).astype(np.float16)            # [D, B]
    w_all = np.concatenate(
        [np.asarray(block_W, np.float32), np.asarray(out_W, np.float32)[None]], axis=0
    ).astype(np.float16)                                          # [NL, K, D, D]
    b_all = np.concatenate(
        [np.asarray(block_b, np.float32), np.asarray(out_b, np.float32)[None]], axis=0
    ).astype(np.float16)                                          # [NL, K, D]
    gw = np.asarray(gate_W, np.float32).astype(np.float16)        # [D, K]
    gb = np.asarray(gate_b, np.float32).astype(np.float16).reshape(1, K)
    in_maps = []
    for c in range(N_CORES):
        in_maps.append(
            {
                "xT": np.ascontiguousarray(xT[:, c * BS : (c + 1) * BS]),
                "w": w_all,
                "bb": b_all,
                "gw": gw,
                "gbc": gb.reshape(K, 1),
            }
        )
    return in_maps


def _assemble(results):
    parts = [np.asarray(results[c]["outT"], np.float32).T for c in range(N_CORES)]
    return np.ascontiguousarray(np.concatenate(parts, axis=0))


def kernel(x, gate_W, gate_b, block_W, block_b, out_W, out_b):
    runner = _get_runner()
    in_maps = _make_in_maps(x, gate_W, gate_b, block_W, block_b, out_W, out_b)
    dev_in = runner.device_inputs(in_maps)
    outs = runner.run(dev_in)
    return _assemble(runner.to_maps(outs))


def bench(x, gate_W, gate_b, block_W, block_b, out_W, out_b, iters=20):
    """Returns (output, per_iteration_ns) — steady-state pipelined device time."""
    import time as _time

    runner = _get_runner()
    in_maps = _make_in_maps(x, gate_W, gate_b, block_W, block_b, out_W, out_b)
    dev_in = runner.device_inputs(in_maps)
    outs = runner.run(dev_in)  # warm-up + compile
    for o in outs:
        o.block_until_ready()
    t0 = _time.perf_counter()
    all_outs = [runner.run(dev_in) for _ in range(iters)]
    for outs_i in all_outs:
        for o in outs_i:
            o.block_until_ready()
    t1 = _time.perf_counter()
    per_iter_ns = (t1 - t0) / iters * 1e9
    return _assemble(runner.to_maps(all_outs[-1])), per_iter_ns


# revision 5
# speedup vs baseline: 1.0890x; 1.0890x over previous
"""MoIE (mixture of implicit experts) Trainium2 kernel.

Math (per reference):
    alpha = softmax(x @ gate_W + gate_b)                    # (B, K)
    h = x
    for l in 0..3:  h = relu(sum_k alpha_k * (h @ W[l,k] + b[l,k]))
    out = sum_k alpha_k * (h @ out_W[k] + out_b[k])

Strategy (v2 — o2-major sweeps, split PSUM pools):
  - Data-parallel: shard B=32768 tokens over 8 cores (4096 each); replicate
    the small weights. No collectives.
  - Feature-major on device: activations live as hT [D(part), T(free)] so
    chained matmuls need no activation transposes.
  - alpha folded into the *moving* operand: rhs_k = hT * bcast(alphaT[k]);
    PSUM accumulates over experts and contraction chunks; the per-expert
    bias enters as a tiny alphaT-contraction matmul; the gate bias enters
    as the per-partition bias of the exp() activation.
  - T=1024 tokens per tile: a z accumulator [128,1024]f32 is 2 PSUM banks,
    so zpool holds 3 buffers (6 banks) and the gate gets its own 2-bank
    pool — the pipelined gate no longer serializes the next layer's PSUM
    allocation (the 427us build shared one 2-buffer pool for both).
  - o2-major layer sweeps: all (k,i2) expert matmuls for output-half 0
    accumulate + evacuate BEFORE the o2=1 sweep runs. z[o2=0] therefore
    finishes mid-layer and its relu/evac + the next layer's DVE rhs
    production hide under ~7us of o2=1 matmuls (the 427us build finished
    both halves at the layer end, exposing the evac->rhs chain).
  - rhs tiles live across both o2 sweeps: rpool bufs=24 (2KB/partition
    each) so the DVE runs a full layer ahead.
  - fp16 matmul path, fp32 PSUM; fp16 output bounce.
  - LDWEIGHTS dedupe within same-stationary groups (PE stream pinned to
    program order); measured 1055/1062 LDW hidden under matmuls.
  - Software-pipelined gating: tile ti's layer pass carries tile ti+1's
    gate (cyclic across reps at the last tile); with NT=4 the next-rep
    broadcast lands ~3 layer-tiles before the For_i barrier.
"""

import sys

if "/opt/trn_rl_repo" not in sys.path:
    sys.path.insert(0, "/opt/trn_rl_repo")

import numpy as np

import concourse.bass as bass
import concourse.bass_isa as bass_isa
import concourse.tile as tile
import concourse.mybir as mybir
from concourse import bacc
from concourse.bass import _add_dep_helper
from concourse.bass_utils import run_bass_kernel_spmd

N_CORES = 8
B, D, K, L = 32768, 256, 8, 4
NL = L + 1                  # 4 hidden blocks + output block
BS = B // N_CORES           # 4096 tokens per core
T = 1024                    # tokens per on-chip tile
NT = BS // T                # tiles per core
SEG = 512                   # f32 PSUM bank = 512 elements
NSEG = T // SEG
F16 = mybir.dt.float16
F32 = mybir.dt.float32
AF = mybir.ActivationFunctionType
_RPOOL_BUFS = 24
_APOOL_BUFS = 2
_ZPOOL_BUFS = 3
_GPOOL_BUFS = 2
_LDW_DEDUP = True


class _MMEmitter:
    """Emit matmuls, tracking which ones share a stationary operand with the
    immediately preceding matmul. Tile's legalizer splits every InstMatmult
    into InstLdweights + InstMatmult; `_dedupe_ldweights` later deletes the
    redundant loads for the marked matmuls. A nosync dep chain pins the PE
    stream to program order so a dedup'd matmul can never observe a foreign
    group's weights."""

    def __init__(self, nc):
        self.nc = nc
        self.key = None
        self.prev = None
        self.skip_names = set()

    def mm(self, key, out, lhsT, rhs, start, stop):
        bi = self.nc.tensor.matmul(out, lhsT, rhs, start=start, stop=stop)
        if _LDW_DEDUP:
            if self.prev is not None:
                _add_dep_helper(
                    bi.ins, self.prev, sync=False, reason="pe-program-order"
                )
            if key is not None and key == self.key:
                self.skip_names.add(bi.ins.name)
        self.key = key
        self.prev = bi.ins
        return bi


def _dedupe_ldweights(nc, skip_names):
    """Remove the InstLdweights preceding each marked matmul (same stationary
    as the previous matmul, PE stream pinned to program order). Waits are
    moved onto the matmul; dependency edges are merged / remapped."""
    removed = {}
    for b in nc.m.functions[0].blocks:
        insts = list(b.instructions)
        keep = [True] * len(insts)
        for idx, ins in enumerate(insts):
            if not (isinstance(ins, mybir.InstMatmult) and ins.name in skip_names):
                continue
            j = idx - 1
            lw = None
            while j >= 0:
                pj = insts[j]
                if isinstance(pj, mybir.InstLdweights):
                    if keep[j]:
                        lw = pj
                    break
                if isinstance(pj, mybir.InstMatmult):
                    break
                j -= 1
            if lw is None:
                continue
            si = lw.sync_info
            if si is not None and len(si.on_update) > 0:
                continue  # LDW signals a semaphore: leave it alone
            if si is not None and len(si.on_wait) > 0:
                msi = ins.sync_info
                waits = list(si.on_wait) + (
                    list(msi.on_wait) if msi is not None else []
                )
                upds = list(msi.on_update) if msi is not None else []
                ins.sync_info = mybir.SyncInfo(on_wait=waits, on_update=upds)
            ins.merge_dependencies_from(lw)
            keep[j] = False
            removed[lw.name] = ins.name
        if not all(keep):
            b.instructions = [i for i, k in zip(insts, keep) if k]
    if removed:
        for b in nc.m.functions[0].blocks:
            for i in b.instructions:
                i.remap_dependency_names(removed)
    return len(removed)


def _build_kernel(reps=1):
    nc = bacc.Bacc(
        "TRN2",
        target_bir_lowering=False,
        debug=False,
        enable_asserts=False,
        num_devices=N_CORES,
    )
    xT = nc.dram_tensor("xT", [D, BS], F16, kind="ExternalInput").ap()
    w = nc.dram_tensor("w", [NL, K, D, D], F16, kind="ExternalInput").ap()
    bb = nc.dram_tensor("bb", [NL, K, D], F16, kind="ExternalInput").ap()
    gw = nc.dram_tensor("gw", [D, K], F16, kind="ExternalInput").ap()
    gbc = nc.dram_tensor("gbc", [K, 1], F16, kind="ExternalInput").ap()
    outT = nc.dram_tensor("outT", [D, BS], F16, kind="ExternalOutput").ap()

    with tile.TileContext(nc) as tc:
        em = _body(nc, tc, xT, w, bb, gw, gbc, outT, reps)
    if _LDW_DEDUP and em is not None:
        n = _dedupe_ldweights(nc, em.skip_names)
        assert n == len(em.skip_names), (n, len(em.skip_names))
    nc.compile()
    return nc


def _body(nc, tc, xT, w, bb, gw, gbc_d, outT, reps=1):
    with (
        tc.tile_pool(name="cpool", bufs=1) as cpool,
        tc.tile_pool(name="xpool", bufs=2 * NT) as xpool,
        tc.tile_pool(name="hpool", bufs=6) as hpool,
        tc.tile_pool(name="rpool", bufs=_RPOOL_BUFS) as rpool,
        tc.tile_pool(name="apool", bufs=_APOOL_BUFS) as apool,
        tc.tile_pool(name="spool", bufs=2) as spool,
        tc.tile_pool(name="opool", bufs=2) as opool,
        tc.tile_pool(name="dpool", bufs=2, space=bass.MemorySpace.DRAM) as dpool,
        tc.tile_pool(name="zpool", bufs=_ZPOOL_BUFS, space=bass.MemorySpace.PSUM) as zpool,
        tc.tile_pool(name="gpool", bufs=_GPOOL_BUFS, space=bass.MemorySpace.PSUM) as gpool,
    ):
        # ---- small constants first (the HWDGE queue is FIFO: keep the
        # gate/bias/x transfers ahead of the 5MB weight stream) ----
        gwt = cpool.tile([128, 2 * K], F16, name="gwt")
        for i2 in range(2):
            nc.sync.dma_start(
                gwt[:, i2 * K : (i2 + 1) * K], gw[i2 * 128 : (i2 + 1) * 128, :]
            )
        gbc = cpool.tile([K, 1], F16, name="gbc")
        nc.sync.dma_start(gbc[:], gbc_d[:])
        bt = cpool.tile([K, NL * D], F16, name="bt")
        ones8x8 = cpool.tile([K, K], F16, name="ones8x8")
        nc.vector.memset(ones8x8[:], 1.0)
        wt = cpool.tile([128, NL * K * 2 * D], F16, name="wt")

        def load_weights():
            for l in range(NL):
                nc.sync.dma_start(bt[:, l * D : (l + 1) * D], bb[l])
            for l in range(NL):
                for k in range(K):
                    for i2 in range(2):
                        off = ((l * K + k) * 2 + i2) * D
                        nc.sync.dma_start(
                            wt[:, off : off + D],
                            w[l, k, i2 * 128 : (i2 + 1) * 128, :],
                        )

        def wslice(l, k, i2, o2):
            base = ((l * K + k) * 2 + i2) * D + o2 * 128
            return wt[:, base : base + 128]

        em = _MMEmitter(nc)

        if reps > 1:
            # steady-state benchmarking variant: weights resident across reps
            load_weights()

        hs, alphaTs, abcs = {}, {}, {}
        emit_seq = [0]

        def load_x(ti):
            sq = emit_seq[0]
            t0 = ti * T
            h = []
            for i2 in range(2):
                ht = xpool.tile([128, T], F16, tag="x", name=f"x_{ti}_{i2}_{sq}")
                for s in range(NSEG):
                    sl = slice(s * SEG, (s + 1) * SEG)
                    nc.sync.dma_start(
                        ht[:, sl],
                        xT[i2 * 128 : (i2 + 1) * 128, t0 + s * SEG : t0 + (s + 1) * SEG],
                    )
                h.append(ht)
            hs[ti] = h

        def gate_partA(ti):
            """Gate logits glT[k,t] = gate_W.T @ x per half-tile + exp().
            Emitted right after L0's o2=0 sweep so the exp ACT instruction
            queues ahead of the o2=1 evacuation (the sT8 matmuls, emitted
            after the o2=1 sweep, then find eT ready)."""
            sq = emit_seq[0]
            emit_seq[0] += 1
            h = hs[ti]
            eT = spool.tile([K, T], F16, tag="eT", name=f"eT_{ti}_{sq}", bufs=1)
            gate_stats = [
                (gwt[:, 0:K], h[0]),
                (gwt[:, K : 2 * K], h[1]),
            ]
            nstat = len(gate_stats)
            HT = T // 2
            glTs = []
            for hf in range(2):
                glT = gpool.tile([K, HT], F32, tag="g", name=f"glT_{ti}_{hf}_{sq}")
                for wi, (stat, mov) in enumerate(gate_stats):
                    for s in range(NSEG // 2):
                        sl = slice(s * SEG, (s + 1) * SEG)
                        gsl = slice(hf * HT + s * SEG, hf * HT + (s + 1) * SEG)
                        em.mm(
                            ("gate", wi),
                            glT[:, sl],
                            stat,
                            mov[:, gsl],
                            start=(wi == 0),
                            stop=(wi == nstat - 1),
                        )
                # softmax over the 8 partitions (no max-subtract needed;
                # logits are ~N(0,1) so exp() is safe); gate bias enters as
                # the ACT per-partition bias vector
                hsl = slice(hf * HT, (hf + 1) * HT)
                nc.scalar.activation(eT[:, hsl], glT[:], AF.Exp, bias=gbc[:])
                glTs.append(glT)
            return {"ti": ti, "sq": sq, "eT": eT}

        def gate_partB_mms(st):
            """sum-over-experts matmuls: all-ones [8,8] lhsT -> every row is
            sum_k e_k. Emitted after L0's o2=1 sweep (eT is ready by then)."""
            ti, sq, eT = st["ti"], st["sq"], st["eT"]
            HT = T // 2
            sT8s = []
            for hf in range(2):
                sT8 = gpool.tile([K, HT], F32, tag="g", name=f"sT8_{ti}_{hf}_{sq}")
                for s in range(NSEG // 2):
                    sl = slice(s * SEG, (s + 1) * SEG)
                    esl = slice(hf * HT + s * SEG, hf * HT + (s + 1) * SEG)
                    em.mm(
                        ("ones8",),
                        sT8[:, sl],
                        ones8x8[:],
                        eT[:, esl],
                        start=True,
                        stop=True,
                    )
                sT8s.append(sT8)
            st["sT8s"] = sT8s

        def gate_partB_rest(st):
            """reciprocal + alpha = e/sum + DRAM-bounce broadcast. Emitted
            after L1's rhs production so the DVE FIFO reaches the next
            layer's rhs muls before blocking on the sT8 result."""
            ti, sq, eT = st["ti"], st["sq"], st["eT"]
            HT = T // 2
            r8 = spool.tile([K, T], F16, tag="rT", name=f"r8_{ti}_{sq}", bufs=1)
            with nc.allow_low_precision("fp16 softmax normalizer"):
                for hf in range(2):
                    hsl = slice(hf * HT, (hf + 1) * HT)
                    nc.vector.reciprocal(r8[:, hsl], st["sT8s"][hf][:])
            alphaT = spool.tile([K, T], F16, tag="alphaT", name=f"alphaT_{ti}_{sq}")
            nc.vector.tensor_mul(alphaT[:], eT[:], r8[:])

            # broadcast alphaT rows across all 128 partitions so the DVE can
            # multiply h by alpha_k elementwise: bounce through DRAM (SBUF-
            # source broadcast APs are unsupported), then step-0 DRAM->SBUF
            # broadcast DMAs on the otherwise-idle GpSimd queue + Sync
            abc = apool.tile([128, K * T], F16, tag="abc", name=f"abc_{ti}_{sq}")
            adram = dpool.tile([K, T], F16, tag="adram", name=f"adram_{ti}_{sq}")
            nc.gpsimd.dma_start(adram[:], alphaT[:])
            bengs = [nc.gpsimd, nc.sync]
            for k in range(K):
                bengs[k % len(bengs)].dma_start(
                    abc[:, k * T : (k + 1) * T],
                    adram[k : k + 1, :].broadcast_to([128, T]),
                )
            alphaTs[ti] = alphaT
            abcs[ti] = abc

        def gate_stage(ti):
            """Standalone gate (prologue only)."""
            st = gate_partA(ti)
            gate_partB_mms(st)
            gate_partB_rest(st)
            if ti == 0 and reps == 1:
                load_weights()

        def layers_stage(ti, next_tile=None):
            t0 = ti * T
            h = hs[ti]
            alphaT = alphaTs[ti]
            abc = abcs[ti]
            gate_st = None
            for l in range(NL):
                # rhs production i2-major: the 8 i2=0 tiles depend only on
                # h[0] (evacuated mid-previous-layer) so the DVE never blocks
                # on h[1] (which lands at the previous layer's end)
                rhs = {}
                for i2 in range(2):
                    for k in range(K):
                        rt = rpool.tile(
                            [128, T], F16, tag="rhs", name=f"rhs_{ti}_{l}_{k}_{i2}"
                        )
                        nc.vector.tensor_mul(
                            rt[:], h[i2][:], abc[:, k * T : (k + 1) * T]
                        )
                        rhs[k, i2] = rt
                if l == 1 and gate_st is not None:
                    # gate part B tail sits after this layer's rhs muls in
                    # the DVE FIFO (the reciprocal waits on sT8; anything
                    # emitted after it would stall behind that wait)
                    gate_partB_rest(gate_st)

                newh = []
                for o2 in range(2):
                    zt = zpool.tile([128, T], F32, tag="z", name=f"z_{ti}_{l}_{o2}")
                    # bias first (starts the accumulation group), then all
                    # experts i2-major (matching production); stop on the
                    # last expert per seg
                    for s in range(NSEG):
                        sl = slice(s * SEG, (s + 1) * SEG)
                        em.mm(
                            ("bt", l, o2),
                            zt[:, sl],
                            bt[:, l * D + o2 * 128 : l * D + (o2 + 1) * 128],
                            alphaT[:, sl],
                            start=True,
                            stop=False,
                        )
                    for i2 in range(2):
                        for k in range(K):
                            last = (k == K - 1) and (i2 == 1)
                            for s in range(NSEG):
                                sl = slice(s * SEG, (s + 1) * SEG)
                                em.mm(
                                    ("w", l, k, i2, o2),
                                    zt[:, sl],
                                    wslice(l, k, i2, o2),
                                    rhs[k, i2][:, sl],
                                    start=False,
                                    stop=last,
                                )
                    # immediate evacuation: o2=0's relu runs under the o2=1
                    # sweep, so the next layer's rhs production starts with
                    # ~7us of slack
                    if l < NL - 1:
                        nh = hpool.tile([128, T], F16, tag="h", name=f"h_{ti}_{l}_{o2}")
                        nc.scalar.activation(nh[:], zt[:], AF.Relu)
                        newh.append(nh)
                    else:
                        ot = opool.tile([128, T], F16, tag="o", name=f"out_{ti}_{o2}")
                        nc.scalar.activation(ot[:], zt[:], AF.Copy)
                        nc.sync.dma_start(
                            outT[o2 * 128 : (o2 + 1) * 128, t0 : t0 + T], ot[:]
                        )
                    if l == 0 and next_tile is not None:
                        # next tile's gate: logits+exp after the o2=0 sweep
                        # (exp queues ahead of the o2=1 evac on ACT), sT8
                        # matmuls after the o2=1 sweep (eT ready by then)
                        if o2 == 0:
                            load_x(next_tile)
                            gate_st = gate_partA(next_tile)
                        else:
                            gate_partB_mms(gate_st)
                h = newh

        # prologue: tile 0's gate outside the reps loop; each tile's layers
        # then carry the NEXT tile's gate (cyclic across reps)
        load_x(0)
        gate_stage(0)
        ctx = None
        if reps > 1:
            ctx = tc.For_i(0, reps, 1)
            ctx.__enter__()
        for ti in range(NT):
            nxt = ti + 1 if ti + 1 < NT else (0 if reps > 1 else None)
            layers_stage(ti, next_tile=nxt)

        if ctx is not None:
            ctx.__exit__(None, None, None)
        return em


_NC_CACHE = None


def _get_nc():
    global _NC_CACHE
    if _NC_CACHE is None:
        _NC_CACHE = _build_kernel()
    return _NC_CACHE


class _Runner:
    """Persistent sharded PJRT executable for the bass kernel (compile once,
    run many). Mirrors bass2jax.run_bass_via_pjrt's multi-core branch minus
    buffer donation (the kernel writes every output element)."""

    def __init__(self, nc=None):
        import jax
        from jax.sharding import Mesh, PartitionSpec, NamedSharding
        from jax.experimental.shard_map import shard_map
        from concourse import bass2jax, mybir as _mybir

        self.jax = jax
        if nc is None:
            nc = _get_nc()
        bass2jax.install_neuronx_cc_hook()
        part_name = nc.partition_id_tensor.name if nc.partition_id_tensor else None
        in_names, out_names, out_avals, zero_outs = [], [], [], []
        for alloc in nc.m.functions[0].allocations:
            if not isinstance(alloc, _mybir.MemoryLocationSet):
                continue
            name = alloc.memorylocations[0].name
            if alloc.kind == "ExternalInput":
                if name != part_name:
                    in_names.append(name)
            elif alloc.kind == "ExternalOutput":
                out_names.append(name)
                shape = tuple(alloc.tensor_shape)
                dtype = _mybir.dt.np(alloc.dtype)
                out_avals.append(jax.core.ShapedArray(shape, dtype))
                zero_outs.append(np.zeros(shape, dtype))
        self.in_names, self.out_names, self.out_avals = in_names, out_names, out_avals

        bind_names = in_names + out_names + ([part_name] if part_name else [])

        def _body(*args):
            operands = list(args)
            if part_name is not None:
                operands.append(bass2jax.partition_id_tensor())
            outs = bass2jax._bass_exec_p.bind(
                *operands,
                out_avals=tuple(out_avals),
                in_names=tuple(bind_names),
                out_names=tuple(out_names),
                lowering_input_output_aliases=(),
                sim_require_finite=True,
                sim_require_nnan=True,
                nc=nc,
            )
            return tuple(outs)

        devices = jax.devices()[:N_CORES]
        self.mesh = Mesh(np.asarray(devices), ("core",))
        self.spec = PartitionSpec("core")
        self.sharding = NamedSharding(self.mesh, self.spec)
        n_args = len(in_names) + len(out_names)
        self.fn = jax.jit(
            shard_map(
                _body,
                mesh=self.mesh,
                in_specs=(self.spec,) * n_args,
                out_specs=(self.spec,) * len(out_names),
                check_rep=False,
            ),
            keep_unused=True,
        )
        self.zero_outs = [
            jax.device_put(
                np.zeros((N_CORES * z.shape[0], *z.shape[1:]), z.dtype), self.sharding
            )
            for z in zero_outs
        ]

    def device_inputs(self, in_maps):
        concat = [
            np.concatenate([np.asarray(m[name]) for m in in_maps], axis=0)
            for name in self.in_names
        ]
        return [self.jax.device_put(a, self.sharding) for a in concat]

    def run(self, dev_in):
        outs = self.fn(*dev_in, *self.zero_outs)
        return outs

    def to_maps(self, outs):
        res = []
        for c in range(N_CORES):
            res.append(
                {
                    name: np.asarray(outs[i]).reshape(
                        N_CORES, *self.out_avals[i].shape
                    )[c]
                    for i, name in enumerate(self.out_names)
                }
            )
        return res


_RUNNER = None


def _get_runner():
    global _RUNNER
    if _RUNNER is None:
        _RUNNER = _Runner()
    return _RUNNER


def _make_in_maps(x, gate_W, gate_b, block_W, block_b, out_W, out_b):
    x = np.asarray(x, dtype=np.float32)
    xT = np.ascontiguousarray(x.T).astype(np.float16)            # [D, B]
    w_all = np.concatenate(
        [np.asarray(block_W, np.float32), np.asarray(out_W, np.float32)[None]], axis=0
    ).astype(np.float16)                                          # [NL, K, D, D]
    b_all = np.concatenate(
        [np.asarray(block_b, np.float32), np.asarray(out_b, np.float32)[None]], axis=0
    ).astype(np.float16)                                          # [NL, K, D]
    gw = np.asarray(gate_W, np.float32).astype(np.float16)        # [D, K]
    gb = np.asarray(gate_b, np.float32).astype(np.float16).reshape(1, K)
    in_maps = []
    for c in range(N_CORES):
        in_maps.append(
            {
                "xT": np.ascontiguousarray(xT[:, c * BS : (c + 1) * BS]),
                "w": w_all,
                "bb": b_all,
                "gw": gw,
                "gbc": gb.reshape(K, 1),
            }
        )
    return in_maps


def _assemble(results):
    parts = [np.asarray(results[c]["outT"], np.float32).T for c in range(N_CORES)]
    return np.ascontiguousarray(np.concatenate(parts, axis=0))


def kernel(x, gate_W, gate_b, block_W, block_b, out_W, out_b):
    runner = _get_runner()
    in_maps = _make_in_maps(x, gate_W, gate_b, block_W, block_b, out_W, out_b)
    dev_in = runner.device_inputs(in_maps)
    outs = runner.run(dev_in)
    return _assemble(runner.to_maps(outs))


def bench(x, gate_W, gate_b, block_W, block_b, out_W, out_b, iters=20):
    """Returns (output, per_iteration_ns) — steady-state pipelined device time."""
    import time as _time

    runner = _get_runner()
    in_maps = _make_in_maps(x, gate_W, gate_b, block_W, block_b, out_W, out_b)
    dev_in = runner.device_inputs(in_maps)
    outs = runner.run(dev_in)  # warm-up + compile
    for o in outs:
        o.block_until_ready()
    t0 = _time.perf_counter()
    all_outs = [runner.run(dev_in) for _ in range(iters)]
    for outs_i in all_outs:
        for o in outs_i:
            o.block_until_ready()
    t1 = _time.perf_counter()
    per_iter_ns = (t1 - t0) / iters * 1e9
    return _assemble(runner.to_maps(all_outs[-1])), per_iter_ns


# revision 11
# speedup vs baseline: 1.1205x; 1.0289x over previous
"""MoIE (mixture of implicit experts) Trainium2 kernel.

Math (per reference):
    alpha = softmax(x @ gate_W + gate_b)                    # (B, K)
    h = x
    for l in 0..3:  h = relu(sum_k alpha_k * (h @ W[l,k] + b[l,k]))
    out = sum_k alpha_k * (h @ out_W[k] + out_b[k])

Strategy (v2 — o2-major sweeps, split PSUM pools):
  - Data-parallel: shard B=32768 tokens over 8 cores (4096 each); replicate
    the small weights. No collectives.
  - Feature-major on device: activations live as hT [D(part), T(free)] so
    chained matmuls need no activation transposes.
  - alpha folded into the *moving* operand: rhs_k = hT * bcast(alphaT[k]);
    PSUM accumulates over experts and contraction chunks; the per-expert
    bias enters as a tiny alphaT-contraction matmul; the gate bias enters
    as the per-partition bias of the exp() activation.
  - T=1024 tokens per tile: a z accumulator [128,1024]f32 is 2 PSUM banks,
    so zpool holds 3 buffers (6 banks) and the gate gets its own 2-bank
    pool — the pipelined gate no longer serializes the next layer's PSUM
    allocation (the 427us build shared one 2-buffer pool for both).
  - o2-major layer sweeps: all (k,i2) expert matmuls for output-half 0
    accumulate + evacuate BEFORE the o2=1 sweep runs. z[o2=0] therefore
    finishes mid-layer and its relu/evac + the next layer's DVE rhs
    production hide under ~7us of o2=1 matmuls (the 427us build finished
    both halves at the layer end, exposing the evac->rhs chain).
  - rhs tiles live across both o2 sweeps: rpool bufs=24 (2KB/partition
    each) so the DVE runs a full layer ahead.
  - fp16 matmul path, fp32 PSUM; fp16 output bounce.
  - LDWEIGHTS dedupe within same-stationary groups (PE stream pinned to
    program order); measured 1055/1062 LDW hidden under matmuls.
  - Software-pipelined gating: tile ti's layer pass carries tile ti+1's
    gate (cyclic across reps at the last tile); with NT=4 the next-rep
    broadcast lands ~3 layer-tiles before the For_i barrier.
"""

import sys

if "/opt/trn_rl_repo" not in sys.path:
    sys.path.insert(0, "/opt/trn_rl_repo")

import numpy as np

import concourse.bass as bass
import concourse.bass_isa as bass_isa
import concourse.tile as tile
import concourse.mybir as mybir
from concourse import bacc
from concourse.bass import _add_dep_helper
from concourse.bass_utils import run_bass_kernel_spmd

N_CORES = 8
B, D, K, L = 32768, 256, 8, 4
NL = L + 1                  # 4 hidden blocks + output block
BS = B // N_CORES           # 4096 tokens per core
T = 1024                    # tokens per on-chip tile
NT = BS // T                # tiles per core
SEG = 512                   # f32 PSUM bank = 512 elements
NSEG = T // SEG
F16 = mybir.dt.float16
F32 = mybir.dt.float32
AF = mybir.ActivationFunctionType
_RPOOL_BUFS = 18
_APOOL_BUFS = 2
_ZPOOL_BUFS = 3
_GPOOL_BUFS = 2
_LDW_DEDUP = True


class _MMEmitter:
    """Emit matmuls, tracking which ones share a stationary operand with the
    immediately preceding matmul. Tile's legalizer splits every InstMatmult
    into InstLdweights + InstMatmult; `_dedupe_ldweights` later deletes the
    redundant loads for the marked matmuls. A nosync dep chain pins the PE
    stream to program order so a dedup'd matmul can never observe a foreign
    group's weights."""

    def __init__(self, nc):
        self.nc = nc
        self.key = None
        self.prev = None
        self.skip_names = set()

    def mm(self, key, out, lhsT, rhs, start, stop, tp=None):
        bi = self.nc.tensor.matmul(
            out, lhsT, rhs, start=start, stop=stop, tile_position=tp
        )
        if _LDW_DEDUP:
            if self.prev is not None:
                _add_dep_helper(
                    bi.ins, self.prev, sync=False, reason="pe-program-order"
                )
            if key is not None and key == self.key:
                self.skip_names.add(bi.ins.name)
        self.key = key
        self.prev = bi.ins
        return bi


def _dedupe_ldweights(nc, skip_names):
    """Remove the InstLdweights preceding each marked matmul (same stationary
    as the previous matmul, PE stream pinned to program order). Waits are
    moved onto the matmul; dependency edges are merged / remapped."""
    removed = {}
    for b in nc.m.functions[0].blocks:
        insts = list(b.instructions)
        keep = [True] * len(insts)
        for idx, ins in enumerate(insts):
            if not (isinstance(ins, mybir.InstMatmult) and ins.name in skip_names):
                continue
            j = idx - 1
            lw = None
            while j >= 0:
                pj = insts[j]
                if isinstance(pj, mybir.InstLdweights):
                    if keep[j]:
                        lw = pj
                    break
                if isinstance(pj, mybir.InstMatmult):
                    break
                j -= 1
            if lw is None:
                continue
            si = lw.sync_info
            if si is not None and len(si.on_update) > 0:
                continue  # LDW signals a semaphore: leave it alone
            if si is not None and len(si.on_wait) > 0:
                msi = ins.sync_info
                waits = list(si.on_wait) + (
                    list(msi.on_wait) if msi is not None else []
                )
                upds = list(msi.on_update) if msi is not None else []
                ins.sync_info = mybir.SyncInfo(on_wait=waits, on_update=upds)
            ins.merge_dependencies_from(lw)
            keep[j] = False
            removed[lw.name] = ins.name
        if not all(keep):
            b.instructions = [i for i, k in zip(insts, keep) if k]
    if removed:
        for b in nc.m.functions[0].blocks:
            for i in b.instructions:
                i.remap_dependency_names(removed)
    return len(removed)


def _build_kernel(reps=1):
    nc = bacc.Bacc(
        "TRN2",
        target_bir_lowering=False,
        debug=False,
        enable_asserts=False,
        num_devices=N_CORES,
    )
    xT = nc.dram_tensor("xT", [D, BS], F16, kind="ExternalInput").ap()
    w = nc.dram_tensor("w", [NL, K, D, D], F16, kind="ExternalInput").ap()
    bb = nc.dram_tensor("bb", [NL, K, D], F16, kind="ExternalInput").ap()
    gw = nc.dram_tensor("gw", [D, K], F16, kind="ExternalInput").ap()
    gbc = nc.dram_tensor("gbc", [K, 1], F16, kind="ExternalInput").ap()
    outT = nc.dram_tensor("outT", [D, BS], F16, kind="ExternalOutput").ap()

    with tile.TileContext(nc) as tc:
        em = _body(nc, tc, xT, w, bb, gw, gbc, outT, reps)
    if _LDW_DEDUP and em is not None:
        n = _dedupe_ldweights(nc, em.skip_names)
        assert n == len(em.skip_names), (n, len(em.skip_names))
    nc.compile()
    return nc


def _body(nc, tc, xT, w, bb, gw, gbc_d, outT, reps=1):
    with (
        tc.tile_pool(name="cpool", bufs=1) as cpool,
        tc.tile_pool(name="xpool", bufs=2 * NT) as xpool,
        tc.tile_pool(name="hpool", bufs=6) as hpool,
        tc.tile_pool(name="rpool", bufs=_RPOOL_BUFS) as rpool,
        tc.tile_pool(name="apool", bufs=_APOOL_BUFS) as apool,
        tc.tile_pool(name="spool", bufs=2) as spool,
        tc.tile_pool(name="opool", bufs=2) as opool,
        tc.tile_pool(name="dpool", bufs=2, space=bass.MemorySpace.DRAM) as dpool,
        tc.tile_pool(name="zpool", bufs=_ZPOOL_BUFS, space=bass.MemorySpace.PSUM) as zpool,
        tc.tile_pool(name="gpool", bufs=_GPOOL_BUFS, space=bass.MemorySpace.PSUM) as gpool,
    ):
        # ---- small constants first (the HWDGE queue is FIFO: keep the
        # gate/bias/x transfers ahead of the 5MB weight stream) ----
        gwt = cpool.tile([128, 2 * K], F16, name="gwt")
        for i2 in range(2):
            nc.sync.dma_start(
                gwt[:, i2 * K : (i2 + 1) * K], gw[i2 * 128 : (i2 + 1) * 128, :]
            )
        gbc = cpool.tile([K, 1], F16, name="gbc")
        nc.sync.dma_start(gbc[:], gbc_d[:])
        bt = cpool.tile([K, NL * D], F16, name="bt")
        # second bias copy on partitions 32-39: feeds the o2=1 bias matmul at
        # array rows 32-63 (tile_position=(32,0)) so both o2 bias matmuls run
        # concurrently in separate row groups
        bt40 = cpool.tile([40, NL * D], F16, name="bt40")
        ones8x8 = cpool.tile([K, K], F16, name="ones8x8")
        nc.vector.memset(ones8x8[:], 1.0)
        wt = cpool.tile([128, NL * K * 2 * D], F16, name="wt")

        def load_weights():
            for l in range(NL):
                nc.sync.dma_start(bt[:, l * D : (l + 1) * D], bb[l])
                nc.sync.dma_start(bt40[32:40, l * D : (l + 1) * D], bb[l])
            for l in range(NL):
                for k in range(K):
                    for i2 in range(2):
                        off = ((l * K + k) * 2 + i2) * D
                        nc.sync.dma_start(
                            wt[:, off : off + D],
                            w[l, k, i2 * 128 : (i2 + 1) * 128, :],
                        )

        def wslice(l, k, i2, o2):
            base = ((l * K + k) * 2 + i2) * D + o2 * 128
            return wt[:, base : base + 128]

        em = _MMEmitter(nc)

        if reps > 1:
            # steady-state benchmarking variant: weights resident across reps
            load_weights()

        hs, alphaTs, a40s, abcs = {}, {}, {}, {}
        emit_seq = [0]

        def load_x(ti):
            sq = emit_seq[0]
            t0 = ti * T
            h = []
            for i2 in range(2):
                ht = xpool.tile([128, T], F16, tag="x", name=f"x_{ti}_{i2}_{sq}")
                for s in range(NSEG):
                    sl = slice(s * SEG, (s + 1) * SEG)
                    nc.sync.dma_start(
                        ht[:, sl],
                        xT[i2 * 128 : (i2 + 1) * 128, t0 + s * SEG : t0 + (s + 1) * SEG],
                    )
                h.append(ht)
            hs[ti] = h

        def gate_partA(ti):
            """Gate logits glT[k,t] = gate_W.T @ x per half-tile + exp().
            Emitted right after L0's o2=0 sweep so the exp ACT instruction
            queues ahead of the o2=1 evacuation (the sT8 matmuls, emitted
            after the o2=1 sweep, then find eT ready)."""
            sq = emit_seq[0]
            emit_seq[0] += 1
            h = hs[ti]
            eT = spool.tile([K, T], F16, tag="eT", name=f"eT_{ti}_{sq}", bufs=1)
            gate_stats = [
                (gwt[:, 0:K], h[0]),
                (gwt[:, K : 2 * K], h[1]),
            ]
            nstat = len(gate_stats)
            HT = T // 2
            glTs = []
            for hf in range(2):
                glT = gpool.tile([K, HT], F32, tag="g", name=f"glT_{ti}_{hf}_{sq}")
                for wi, (stat, mov) in enumerate(gate_stats):
                    for s in range(NSEG // 2):
                        sl = slice(s * SEG, (s + 1) * SEG)
                        gsl = slice(hf * HT + s * SEG, hf * HT + (s + 1) * SEG)
                        em.mm(
                            ("gate", wi),
                            glT[:, sl],
                            stat,
                            mov[:, gsl],
                            start=(wi == 0),
                            stop=(wi == nstat - 1),
                        )
                # softmax over the 8 partitions (no max-subtract needed;
                # logits are ~N(0,1) so exp() is safe); gate bias enters as
                # the ACT per-partition bias vector
                hsl = slice(hf * HT, (hf + 1) * HT)
                nc.scalar.activation(eT[:, hsl], glT[:], AF.Exp, bias=gbc[:])
                glTs.append(glT)
            return {"ti": ti, "sq": sq, "eT": eT}

        def gate_partB_mms(st):
            """sum-over-experts matmuls: all-ones [8,8] lhsT -> every row is
            sum_k e_k. Emitted after L0's o2=1 sweep (eT is ready by then)."""
            ti, sq, eT = st["ti"], st["sq"], st["eT"]
            HT = T // 2
            sT8s = []
            for hf in range(2):
                sT8 = gpool.tile([K, HT], F32, tag="g", name=f"sT8_{ti}_{hf}_{sq}")
                for s in range(NSEG // 2):
                    sl = slice(s * SEG, (s + 1) * SEG)
                    esl = slice(hf * HT + s * SEG, hf * HT + (s + 1) * SEG)
                    em.mm(
                        ("ones8",),
                        sT8[:, sl],
                        ones8x8[:],
                        eT[:, esl],
                        start=True,
                        stop=True,
                    )
                sT8s.append(sT8)
            st["sT8s"] = sT8s

        def gate_partB_rest(st):
            """reciprocal + alpha = e/sum + DRAM-bounce broadcast. Emitted
            after L1's rhs production so the DVE FIFO reaches the next
            layer's rhs muls before blocking on the sT8 result."""
            ti, sq, eT = st["ti"], st["sq"], st["eT"]
            HT = T // 2
            r8 = spool.tile([K, T], F16, tag="rT", name=f"r8_{ti}_{sq}", bufs=1)
            with nc.allow_low_precision("fp16 softmax normalizer"):
                for hf in range(2):
                    hsl = slice(hf * HT, (hf + 1) * HT)
                    nc.vector.reciprocal(r8[:, hsl], st["sT8s"][hf][:])
            alphaT = spool.tile([K, T], F16, tag="alphaT", name=f"alphaT_{ti}_{sq}")
            nc.vector.tensor_mul(alphaT[:], eT[:], r8[:])

            # broadcast alphaT rows across all 128 partitions so the DVE can
            # multiply h by alpha_k elementwise: bounce through DRAM (SBUF-
            # source broadcast APs are unsupported), then step-0 DRAM->SBUF
            # broadcast DMAs on the otherwise-idle GpSimd queue + Sync
            abc = apool.tile([128, K * T], F16, tag="abc", name=f"abc_{ti}_{sq}")
            adram = dpool.tile([K, T], F16, tag="adram", name=f"adram_{ti}_{sq}")
            nc.gpsimd.dma_start(adram[:], alphaT[:])
            bengs = [nc.gpsimd, nc.sync]
            for k in range(K):
                bengs[k % len(bengs)].dma_start(
                    abc[:, k * T : (k + 1) * T],
                    adram[k : k + 1, :].broadcast_to([128, T]),
                )
            # alphaT copy on partitions 32-39 for the row-packed o2=1 bias
            a40 = spool.tile([40, T], F16, tag="a32", name=f"a40_{ti}_{sq}")
            nc.gpsimd.dma_start(a40[32:40, :], alphaT[:])
            alphaTs[ti] = alphaT
            a40s[ti] = a40
            abcs[ti] = abc

        def gate_stage(ti):
            """Standalone gate (prologue only)."""
            st = gate_partA(ti)
            gate_partB_mms(st)
            gate_partB_rest(st)
            if ti == 0 and reps == 1:
                load_weights()

        def layers_stage(ti, next_tile=None):
            t0 = ti * T
            h = hs[ti]
            alphaT = alphaTs[ti]
            a40 = a40s[ti]
            abc = abcs[ti]
            gate_st = None
            for l in range(NL):
                # rhs production i2-major (the i2=0 tiles depend only on
                # h[0], evacuated mid-previous-layer) and pair-batched: one
                # DVE op covers 2 experts (abc is expert-contiguous; h enters
                # via a stride-0 repeat dim), amortizing the DVE's 151-cycle
                # per-op overhead: 9.8us/layer instead of 11us.
                rhs = {}
                for i2 in range(2):
                    for kp in range(K // 2):
                        rt = rpool.tile(
                            [128, 2 * T], F16, tag="rhs", name=f"rhs_{ti}_{l}_{kp}_{i2}"
                        )
                        nc.vector.tensor_mul(
                            rt[:].rearrange("p (j t) -> p j t", j=2),
                            h[i2][:].unsqueeze(1).broadcast_to([128, 2, T]),
                            abc[:, 2 * kp * T : (2 * kp + 2) * T].rearrange(
                                "p (j t) -> p j t", j=2
                            ),
                        )
                        rhs[kp, i2] = rt
                if l == 1 and gate_st is not None:
                    # gate part B tail sits after this layer's rhs muls in
                    # the DVE FIFO (the reciprocal waits on sT8; anything
                    # emitted after it would stall behind that wait)
                    gate_partB_rest(gate_st)

                zts = [
                    zpool.tile([128, T], F32, tag="z", name=f"z_{ti}_{l}_{o2}")
                    for o2 in range(2)
                ]
                # both bias matmuls up front: o2=0 in array rows 0-31, o2=1 in
                # rows 32-63 (tile_position) — they run concurrently, halving
                # the bias stream time; each starts its z accumulation group
                for s in range(NSEG):
                    sl = slice(s * SEG, (s + 1) * SEG)
                    em.mm(
                        ("bt", l, 0),
                        zts[0][:, sl],
                        bt[:, l * D : l * D + 128],
                        alphaT[:, sl],
                        start=True,
                        stop=False,
                    )
                for s in range(NSEG):
                    sl = slice(s * SEG, (s + 1) * SEG)
                    em.mm(
                        ("bt", l, 1),
                        zts[1][:, sl],
                        bt40[32:40, l * D + 128 : l * D + 256],
                        a40[32:40, sl],
                        start=True,
                        stop=False,
                        tp=(32, 0),
                    )
                newh = []
                for o2 in range(2):
                    zt = zts[o2]
                    for i2 in range(2):
                        for k in range(K):
                            last = (k == K - 1) and (i2 == 1)
                            for s in range(NSEG):
                                sl = slice((k % 2) * T + s * SEG, (k % 2) * T + (s + 1) * SEG)
                                zsl = slice(s * SEG, (s + 1) * SEG)
                                em.mm(
                                    ("w", l, k, i2, o2),
                                    zt[:, zsl],
                                    wslice(l, k, i2, o2),
                                    rhs[k // 2, i2][:, sl],
                                    start=False,
                                    stop=last,
                                )
                    # immediate evacuation: o2=0's relu runs under the o2=1
                    # sweep, so the next layer's rhs production starts with
                    # ~7us of slack
                    if l < NL - 1:
                        nh = hpool.tile([128, T], F16, tag="h", name=f"h_{ti}_{l}_{o2}")
                        nc.scalar.activation(nh[:], zt[:], AF.Relu)
                        newh.append(nh)
                    else:
                        ot = opool.tile([128, T], F16, tag="o", name=f"out_{ti}_{o2}")
                        nc.scalar.activation(ot[:], zt[:], AF.Copy)
                        nc.sync.dma_start(
                            outT[o2 * 128 : (o2 + 1) * 128, t0 : t0 + T], ot[:]
                        )
                    if l == 0 and next_tile is not None:
                        # next tile's gate: logits+exp after the o2=0 sweep
                        # (exp queues ahead of the o2=1 evac on ACT), sT8
                        # matmuls after the o2=1 sweep (eT ready by then)
                        if o2 == 0:
                            load_x(next_tile)
                            gate_st = gate_partA(next_tile)
                        else:
                            gate_partB_mms(gate_st)
                h = newh

        # prologue: tile 0's gate outside the reps loop; each tile's layers
        # then carry the NEXT tile's gate (cyclic across reps)
        load_x(0)
        gate_stage(0)
        ctx = None
        if reps > 1:
            ctx = tc.For_i(0, reps, 1)
            ctx.__enter__()
        for ti in range(NT):
            nxt = ti + 1 if ti + 1 < NT else (0 if reps > 1 else None)
            layers_stage(ti, next_tile=nxt)

        if ctx is not None:
            ctx.__exit__(None, None, None)
        return em


_NC_CACHE = None


def _get_nc():
    global _NC_CACHE
    if _NC_CACHE is None:
        _NC_CACHE = _build_kernel()
    return _NC_CACHE


class _Runner:
    """Persistent sharded PJRT executable for the bass kernel (compile once,
    run many). Mirrors bass2jax.run_bass_via_pjrt's multi-core branch minus
    buffer donation (the kernel writes every output element)."""

    def __init__(self, nc=None):
        import jax
        from jax.sharding import Mesh, PartitionSpec, NamedSharding
        from jax.experimental.shard_map import shard_map
        from concourse import bass2jax, mybir as _mybir

        self.jax = jax
        if nc is None:
            nc = _get_nc()
        bass2jax.install_neuronx_cc_hook()
        part_name = nc.partition_id_tensor.name if nc.partition_id_tensor else None
        in_names, out_names, out_avals, zero_outs = [], [], [], []
        for alloc in nc.m.functions[0].allocations:
            if not isinstance(alloc, _mybir.MemoryLocationSet):
                continue
            name = alloc.memorylocations[0].name
            if alloc.kind == "ExternalInput":
                if name != part_name:
                    in_names.append(name)
            elif alloc.kind == "ExternalOutput":
                out_names.append(name)
                shape = tuple(alloc.tensor_shape)
                dtype = _mybir.dt.np(alloc.dtype)
                out_avals.append(jax.core.ShapedArray(shape, dtype))
                zero_outs.append(np.zeros(shape, dtype))
        self.in_names, self.out_names, self.out_avals = in_names, out_names, out_avals

        bind_names = in_names + out_names + ([part_name] if part_name else [])

        def _body(*args):
            operands = list(args)
            if part_name is not None:
                operands.append(bass2jax.partition_id_tensor())
            outs = bass2jax._bass_exec_p.bind(
                *operands,
                out_avals=tuple(out_avals),
                in_names=tuple(bind_names),
                out_names=tuple(out_names),
                lowering_input_output_aliases=(),
                sim_require_finite=True,
                sim_require_nnan=True,
                nc=nc,
            )
            return tuple(outs)

        devices = jax.devices()[:N_CORES]
        self.mesh = Mesh(np.asarray(devices), ("core",))
        self.spec = PartitionSpec("core")
        self.sharding = NamedSharding(self.mesh, self.spec)
        n_args = len(in_names) + len(out_names)
        self.fn = jax.jit(
            shard_map(
                _body,
                mesh=self.mesh,
                in_specs=(self.spec,) * n_args,
                out_specs=(self.spec,) * len(out_names),
                check_rep=False,
            ),
            keep_unused=True,
        )
        self.zero_outs = [
            jax.device_put(
                np.zeros((N_CORES * z.shape[0], *z.shape[1:]), z.dtype), self.sharding
            )
            for z in zero_outs
        ]

    def device_inputs(self, in_maps):
        concat = [
            np.concatenate([np.asarray(m[name]) for m in in_maps], axis=0)
            for name in self.in_names
        ]
        return [self.jax.device_put(a, self.sharding) for a in concat]

    def run(self, dev_in):
        outs = self.fn(*dev_in, *self.zero_outs)
        return outs

    def to_maps(self, outs):
        res = []
        for c in range(N_CORES):
            res.append(
                {
                    name: np.asarray(outs[i]).reshape(
                        N_CORES, *self.out_avals[i].shape
                    )[c]
                    for i, name in enumerate(self.out_names)
                }
            )
        return res


_RUNNER = None


def _get_runner():
    global _RUNNER
    if _RUNNER is None:
        _RUNNER = _Runner()
    return _RUNNER


def _make_in_maps(x, gate_W, gate_b, block_W, block_b, out_W, out_b):
    x = np.asarray(x, dtype=np.float32)
    xT = np.ascontiguousarray(x.T).astype(np.float16)            # [D, B]
    w_all = np.concatenate(
        [np.asarray(block_W, np.float32), np.asarray(out_W, np.float32)[None]], axis=0
    ).astype(np.float16)                                          # [NL, K, D, D]
    b_all = np.concatenate(
        [np.asarray(block_b, np.float32), np.asarray(out_b, np.float32)[None]], axis=0
    ).astype(np.float16)                                          # [NL, K, D]
    gw = np.asarray(gate_W, np.float32).astype(np.float16)        # [D, K]
    gb = np.asarray(gate_b, np.float32).astype(np.float16).reshape(1, K)
    in_maps = []
    for c in range(N_CORES):
        in_maps.append(
            {
                "xT": np.ascontiguousarray(xT[:, c * BS : (c + 1) * BS]),
                "w": w_all,
                "bb": b_all,
                "gw": gw,
                "gbc": gb.reshape(K, 1),
            }
        )
    return in_maps


def _assemble(results):
    parts = [np.asarray(results[c]["outT"], np.float32).T for c in range(N_CORES)]
    return np.ascontiguousarray(np.concatenate(parts, axis=0))


def kernel(x, gate_W, gate_b, block_W, block_b, out_W, out_b):
    runner = _get_runner()
    in_maps = _make_in_maps(x, gate_W, gate_b, block_W, block_b, out_W, out_b)
    dev_in = runner.device_inputs(in_maps)
    outs = runner.run(dev_in)
    return _assemble(runner.to_maps(outs))


def bench(x, gate_W, gate_b, block_W, block_b, out_W, out_b, iters=20):
    """Returns (output, per_iteration_ns) — steady-state pipelined device time."""
    import time as _time

    runner = _get_runner()
    in_maps = _make_in_maps(x, gate_W, gate_b, block_W, block_b, out_W, out_b)
    dev_in = runner.device_inputs(in_maps)
    outs = runner.run(dev_in)  # warm-up + compile
    for o in outs:
        o.block_until_ready()
    t0 = _time.perf_counter()
    all_outs = [runner.run(dev_in) for _ in range(iters)]
    for outs_i in all_outs:
        for o in outs_i:
            o.block_until_ready()
    t1 = _time.perf_counter()
    per_iter_ns = (t1 - t0) / iters * 1e9
    return _assemble(runner.to_maps(all_outs[-1])), per_iter_ns


# revision 13
# speedup vs baseline: 1.1299x; 1.0084x over previous
"""MoIE (mixture of implicit experts) Trainium2 kernel.

Math (per reference):
    alpha = softmax(x @ gate_W + gate_b)                    # (B, K)
    h = x
    for l in 0..3:  h = relu(sum_k alpha_k * (h @ W[l,k] + b[l,k]))
    out = sum_k alpha_k * (h @ out_W[k] + out_b[k])

Strategy (v2 — o2-major sweeps, split PSUM pools):
  - Data-parallel: shard B=32768 tokens over 8 cores (4096 each); replicate
    the small weights. No collectives.
  - Feature-major on device: activations live as hT [D(part), T(free)] so
    chained matmuls need no activation transposes.
  - alpha folded into the *moving* operand: rhs_k = hT * bcast(alphaT[k]);
    PSUM accumulates over experts and contraction chunks; the per-expert
    bias enters as a tiny alphaT-contraction matmul; the gate bias enters
    as the per-partition bias of the exp() activation.
  - T=1024 tokens per tile: a z accumulator [128,1024]f32 is 2 PSUM banks,
    so zpool holds 3 buffers (6 banks) and the gate gets its own 2-bank
    pool — the pipelined gate no longer serializes the next layer's PSUM
    allocation (the 427us build shared one 2-buffer pool for both).
  - o2-major layer sweeps: all (k,i2) expert matmuls for output-half 0
    accumulate + evacuate BEFORE the o2=1 sweep runs. z[o2=0] therefore
    finishes mid-layer and its relu/evac + the next layer's DVE rhs
    production hide under ~7us of o2=1 matmuls (the 427us build finished
    both halves at the layer end, exposing the evac->rhs chain).
  - rhs tiles live across both o2 sweeps: rpool bufs=24 (2KB/partition
    each) so the DVE runs a full layer ahead.
  - fp16 matmul path, fp32 PSUM; fp16 output bounce.
  - LDWEIGHTS dedupe within same-stationary groups (PE stream pinned to
    program order); measured 1055/1062 LDW hidden under matmuls.
  - Software-pipelined gating: tile ti's layer pass carries tile ti+1's
    gate (cyclic across reps at the last tile); with NT=4 the next-rep
    broadcast lands ~3 layer-tiles before the For_i barrier.
"""

import sys

if "/opt/trn_rl_repo" not in sys.path:
    sys.path.insert(0, "/opt/trn_rl_repo")

import numpy as np

import concourse.bass as bass
import concourse.bass_isa as bass_isa
import concourse.tile as tile
import concourse.mybir as mybir
from concourse import bacc
from concourse.bass import _add_dep_helper
from concourse.bass_utils import run_bass_kernel_spmd

N_CORES = 8
B, D, K, L = 32768, 256, 8, 4
NL = L + 1                  # 4 hidden blocks + output block
BS = B // N_CORES           # 4096 tokens per core
T = 1024                    # tokens per on-chip tile
NT = BS // T                # tiles per core
SEG = 512                   # f32 PSUM bank = 512 elements
NSEG = T // SEG
F16 = mybir.dt.float16
F32 = mybir.dt.float32
AF = mybir.ActivationFunctionType
_RPOOL_BUFS = 18
_APOOL_BUFS = 2
_ZPOOL_BUFS = 3
_GPOOL_BUFS = 2
_LDW_DEDUP = True


class _MMEmitter:
    """Emit matmuls, tracking which ones share a stationary operand with the
    immediately preceding matmul. Tile's legalizer splits every InstMatmult
    into InstLdweights + InstMatmult; `_dedupe_ldweights` later deletes the
    redundant loads for the marked matmuls. A nosync dep chain pins the PE
    stream to program order so a dedup'd matmul can never observe a foreign
    group's weights."""

    def __init__(self, nc):
        self.nc = nc
        self.key = None
        self.prev = None
        self.skip_names = set()

    def mm(self, key, out, lhsT, rhs, start, stop, tp=None):
        bi = self.nc.tensor.matmul(
            out, lhsT, rhs, start=start, stop=stop, tile_position=tp
        )
        if _LDW_DEDUP:
            if self.prev is not None:
                _add_dep_helper(
                    bi.ins, self.prev, sync=False, reason="pe-program-order"
                )
            if key is not None and key == self.key:
                self.skip_names.add(bi.ins.name)
        self.key = key
        self.prev = bi.ins
        return bi


def _dedupe_ldweights(nc, skip_names):
    """Remove the InstLdweights preceding each marked matmul (same stationary
    as the previous matmul, PE stream pinned to program order). Waits are
    moved onto the matmul; dependency edges are merged / remapped."""
    removed = {}
    for b in nc.m.functions[0].blocks:
        insts = list(b.instructions)
        keep = [True] * len(insts)
        for idx, ins in enumerate(insts):
            if not (isinstance(ins, mybir.InstMatmult) and ins.name in skip_names):
                continue
            j = idx - 1
            lw = None
            while j >= 0:
                pj = insts[j]
                if isinstance(pj, mybir.InstLdweights):
                    if keep[j]:
                        lw = pj
                    break
                if isinstance(pj, mybir.InstMatmult):
                    break
                j -= 1
            if lw is None:
                continue
            si = lw.sync_info
            if si is not None and len(si.on_update) > 0:
                continue  # LDW signals a semaphore: leave it alone
            if si is not None and len(si.on_wait) > 0:
                msi = ins.sync_info
                waits = list(si.on_wait) + (
                    list(msi.on_wait) if msi is not None else []
                )
                upds = list(msi.on_update) if msi is not None else []
                ins.sync_info = mybir.SyncInfo(on_wait=waits, on_update=upds)
            ins.merge_dependencies_from(lw)
            keep[j] = False
            removed[lw.name] = ins.name
        if not all(keep):
            b.instructions = [i for i, k in zip(insts, keep) if k]
    if removed:
        for b in nc.m.functions[0].blocks:
            for i in b.instructions:
                i.remap_dependency_names(removed)
    return len(removed)


def _build_kernel(reps=1):
    nc = bacc.Bacc(
        "TRN2",
        target_bir_lowering=False,
        debug=False,
        enable_asserts=False,
        num_devices=N_CORES,
    )
    xT = nc.dram_tensor("xT", [D, BS], F16, kind="ExternalInput").ap()
    w = nc.dram_tensor("w", [NL, K, D, D], F16, kind="ExternalInput").ap()
    bb = nc.dram_tensor("bb", [NL, K, D], F16, kind="ExternalInput").ap()
    gw = nc.dram_tensor("gw", [D, K], F16, kind="ExternalInput").ap()
    gbc = nc.dram_tensor("gbc", [K, 1], F16, kind="ExternalInput").ap()
    outT = nc.dram_tensor("outT", [D, BS], F16, kind="ExternalOutput").ap()

    with tile.TileContext(nc) as tc:
        em = _body(nc, tc, xT, w, bb, gw, gbc, outT, reps)
    if _LDW_DEDUP and em is not None:
        n = _dedupe_ldweights(nc, em.skip_names)
        assert n == len(em.skip_names), (n, len(em.skip_names))
    nc.compile()
    return nc


def _body(nc, tc, xT, w, bb, gw, gbc_d, outT, reps=1):
    with (
        tc.tile_pool(name="cpool", bufs=1) as cpool,
        tc.tile_pool(name="xpool", bufs=2 * NT) as xpool,
        tc.tile_pool(name="hpool", bufs=6) as hpool,
        tc.tile_pool(name="rpool", bufs=_RPOOL_BUFS) as rpool,
        tc.tile_pool(name="apool", bufs=_APOOL_BUFS) as apool,
        tc.tile_pool(name="spool", bufs=2) as spool,
        tc.tile_pool(name="opool", bufs=2) as opool,
        tc.tile_pool(name="dpool", bufs=2, space=bass.MemorySpace.DRAM) as dpool,
        tc.tile_pool(name="zpool", bufs=_ZPOOL_BUFS, space=bass.MemorySpace.PSUM) as zpool,
        tc.tile_pool(name="gpool", bufs=_GPOOL_BUFS, space=bass.MemorySpace.PSUM) as gpool,
    ):
        # ---- small constants first (the HWDGE queue is FIFO: keep the
        # gate/bias/x transfers ahead of the 5MB weight stream) ----
        gwt = cpool.tile([128, 2 * K], F16, name="gwt")
        for i2 in range(2):
            nc.sync.dma_start(
                gwt[:, i2 * K : (i2 + 1) * K], gw[i2 * 128 : (i2 + 1) * 128, :]
            )
        gbc = cpool.tile([K, 1], F16, name="gbc")
        nc.sync.dma_start(gbc[:], gbc_d[:])
        bt = cpool.tile([K, NL * D], F16, name="bt")
        # second bias copy on partitions 32-39: feeds the o2=1 bias matmul at
        # array rows 32-63 (tile_position=(32,0)) so both o2 bias matmuls run
        # concurrently in separate row groups
        bt40 = cpool.tile([40, NL * D], F16, name="bt40")
        ones8x8 = cpool.tile([K, K], F16, name="ones8x8")
        nc.vector.memset(ones8x8[:], 1.0)
        wt = cpool.tile([128, NL * K * 2 * D], F16, name="wt")

        def load_weights():
            for l in range(NL):
                nc.sync.dma_start(bt[:, l * D : (l + 1) * D], bb[l])
                nc.sync.dma_start(bt40[32:40, l * D : (l + 1) * D], bb[l])
            for l in range(NL):
                for k in range(K):
                    for i2 in range(2):
                        off = ((l * K + k) * 2 + i2) * D
                        nc.sync.dma_start(
                            wt[:, off : off + D],
                            w[l, k, i2 * 128 : (i2 + 1) * 128, :],
                        )

        def wslice(l, k, i2, o2):
            base = ((l * K + k) * 2 + i2) * D + o2 * 128
            return wt[:, base : base + 128]

        em = _MMEmitter(nc)

        if reps > 1:
            # steady-state benchmarking variant: weights resident across reps
            load_weights()

        hs, alphaTs, a40s, abcs = {}, {}, {}, {}
        emit_seq = [0]

        def load_x(ti):
            sq = emit_seq[0]
            t0 = ti * T
            h = []
            for i2 in range(2):
                ht = xpool.tile([128, T], F16, tag="x", name=f"x_{ti}_{i2}_{sq}")
                for s in range(NSEG):
                    sl = slice(s * SEG, (s + 1) * SEG)
                    nc.sync.dma_start(
                        ht[:, sl],
                        xT[i2 * 128 : (i2 + 1) * 128, t0 + s * SEG : t0 + (s + 1) * SEG],
                    )
                h.append(ht)
            hs[ti] = h

        def gate_partA(ti):
            """Gate logits glT[k,t] = gate_W.T @ x per half-tile + exp().
            Emitted right after L0's o2=0 sweep so the exp ACT instruction
            queues ahead of the o2=1 evacuation (the sT8 matmuls, emitted
            after the o2=1 sweep, then find eT ready)."""
            sq = emit_seq[0]
            emit_seq[0] += 1
            h = hs[ti]
            eT = spool.tile([K, T], F16, tag="eT", name=f"eT_{ti}_{sq}", bufs=1)
            gate_stats = [
                (gwt[:, 0:K], h[0]),
                (gwt[:, K : 2 * K], h[1]),
            ]
            nstat = len(gate_stats)
            HT = T // 2
            glTs = []
            for hf in range(2):
                glT = gpool.tile([K, HT], F32, tag="g", name=f"glT_{ti}_{hf}_{sq}")
                for wi, (stat, mov) in enumerate(gate_stats):
                    for s in range(NSEG // 2):
                        sl = slice(s * SEG, (s + 1) * SEG)
                        gsl = slice(hf * HT + s * SEG, hf * HT + (s + 1) * SEG)
                        em.mm(
                            ("gate", wi),
                            glT[:, sl],
                            stat,
                            mov[:, gsl],
                            start=(wi == 0),
                            stop=(wi == nstat - 1),
                        )
                # softmax over the 8 partitions (no max-subtract needed;
                # logits are ~N(0,1) so exp() is safe); gate bias enters as
                # the ACT per-partition bias vector
                hsl = slice(hf * HT, (hf + 1) * HT)
                nc.scalar.activation(eT[:, hsl], glT[:], AF.Exp, bias=gbc[:])
                glTs.append(glT)
            return {"ti": ti, "sq": sq, "eT": eT}

        def gate_partB_mms(st):
            """sum-over-experts matmuls: all-ones [8,8] lhsT -> every row is
            sum_k e_k. Emitted after L0's o2=1 sweep (eT is ready by then)."""
            ti, sq, eT = st["ti"], st["sq"], st["eT"]
            HT = T // 2
            sT8s = []
            for hf in range(2):
                sT8 = gpool.tile([K, HT], F32, tag="g", name=f"sT8_{ti}_{hf}_{sq}")
                for s in range(NSEG // 2):
                    sl = slice(s * SEG, (s + 1) * SEG)
                    esl = slice(hf * HT + s * SEG, hf * HT + (s + 1) * SEG)
                    em.mm(
                        ("ones8",),
                        sT8[:, sl],
                        ones8x8[:],
                        eT[:, esl],
                        start=True,
                        stop=True,
                    )
                sT8s.append(sT8)
            st["sT8s"] = sT8s

        def gate_partB_rest(st):
            """reciprocal + alpha = e/sum + DRAM-bounce broadcast. Emitted
            after L1's rhs production so the DVE FIFO reaches the next
            layer's rhs muls before blocking on the sT8 result."""
            ti, sq, eT = st["ti"], st["sq"], st["eT"]
            HT = T // 2
            r8 = spool.tile([K, T], F16, tag="rT", name=f"r8_{ti}_{sq}", bufs=1)
            with nc.allow_low_precision("fp16 softmax normalizer"):
                for hf in range(2):
                    hsl = slice(hf * HT, (hf + 1) * HT)
                    nc.vector.reciprocal(r8[:, hsl], st["sT8s"][hf][:])
            alphaT = spool.tile([K, T], F16, tag="alphaT", name=f"alphaT_{ti}_{sq}")
            nc.vector.tensor_mul(alphaT[:], eT[:], r8[:])

            # broadcast alphaT rows across all 128 partitions so the DVE can
            # multiply h by alpha_k elementwise: bounce through DRAM (SBUF-
            # source broadcast APs are unsupported), then step-0 DRAM->SBUF
            # broadcast DMAs on the otherwise-idle GpSimd queue + Sync
            abc = apool.tile([128, K * T], F16, tag="abc", name=f"abc_{ti}_{sq}")
            adram = dpool.tile([K, T], F16, tag="adram", name=f"adram_{ti}_{sq}")
            nc.gpsimd.dma_start(adram[:], alphaT[:])
            bengs = [nc.gpsimd, nc.sync]
            for k in range(K):
                bengs[k % len(bengs)].dma_start(
                    abc[:, k * T : (k + 1) * T],
                    adram[k : k + 1, :].broadcast_to([128, T]),
                )
            # alphaT copy on partitions 32-39 for the row-packed o2=1 bias
            a40 = spool.tile([40, T], F16, tag="a32", name=f"a40_{ti}_{sq}")
            nc.gpsimd.dma_start(a40[32:40, :], alphaT[:])
            alphaTs[ti] = alphaT
            a40s[ti] = a40
            abcs[ti] = abc

        def gate_stage(ti):
            """Standalone gate (prologue only)."""
            st = gate_partA(ti)
            gate_partB_mms(st)
            gate_partB_rest(st)
            if ti == 0 and reps == 1:
                load_weights()

        def layers_stage(ti, next_tile=None):
            t0 = ti * T
            h = hs[ti]
            alphaT = alphaTs[ti]
            a40 = a40s[ti]
            abc = abcs[ti]
            gate_st = None
            pending_z1 = None
            for l in range(NL):
                # rhs production i2-major (the i2=0 tiles depend only on
                # h[0], evacuated mid-previous-layer) and pair-batched: one
                # DVE op covers 2 experts (abc is expert-contiguous; h enters
                # via a stride-0 repeat dim), amortizing the DVE's 151-cycle
                # per-op overhead: 9.8us/layer instead of 11us.
                rhs = {}
                for i2 in range(2):
                    if i2 == 1 and pending_z1 is not None:
                        # previous layer's o2=1 relu runs on the DVE itself,
                        # placed right before its consumers: no cross-engine
                        # hop on the z1-stop -> h[1] -> rhs critical chain,
                        # and the i2=0 production above stays unblocked
                        zprev, nhprev = pending_z1
                        for s in range(NSEG):
                            ssl = slice(s * SEG, (s + 1) * SEG)
                            nc.vector.tensor_relu(nhprev[:, ssl], zprev[:, ssl])
                        pending_z1 = None
                    for kp in range(K // 2):
                        rt = rpool.tile(
                            [128, 2 * T], F16, tag="rhs", name=f"rhs_{ti}_{l}_{kp}_{i2}"
                        )
                        nc.vector.tensor_mul(
                            rt[:].rearrange("p (j t) -> p j t", j=2),
                            h[i2][:].unsqueeze(1).broadcast_to([128, 2, T]),
                            abc[:, 2 * kp * T : (2 * kp + 2) * T].rearrange(
                                "p (j t) -> p j t", j=2
                            ),
                        )
                        rhs[kp, i2] = rt
                if l == 1 and gate_st is not None:
                    # gate part B tail sits after this layer's rhs muls in
                    # the DVE FIFO (the reciprocal waits on sT8; anything
                    # emitted after it would stall behind that wait)
                    gate_partB_rest(gate_st)

                zts = [
                    zpool.tile([128, T], F32, tag="z", name=f"z_{ti}_{l}_{o2}")
                    for o2 in range(2)
                ]
                # both bias matmuls up front: o2=0 in array rows 0-31, o2=1 in
                # rows 32-63 (tile_position) — they run concurrently, halving
                # the bias stream time; each starts its z accumulation group
                for s in range(NSEG):
                    sl = slice(s * SEG, (s + 1) * SEG)
                    em.mm(
                        ("bt", l, 0),
                        zts[0][:, sl],
                        bt[:, l * D : l * D + 128],
                        alphaT[:, sl],
                        start=True,
                        stop=False,
                    )
                for s in range(NSEG):
                    sl = slice(s * SEG, (s + 1) * SEG)
                    em.mm(
                        ("bt", l, 1),
                        zts[1][:, sl],
                        bt40[32:40, l * D + 128 : l * D + 256],
                        a40[32:40, sl],
                        start=True,
                        stop=False,
                        tp=(32, 0),
                    )
                newh = []
                for o2 in range(2):
                    zt = zts[o2]
                    for i2 in range(2):
                        for k in range(K):
                            last = (k == K - 1) and (i2 == 1)
                            for s in range(NSEG):
                                sl = slice((k % 2) * T + s * SEG, (k % 2) * T + (s + 1) * SEG)
                                zsl = slice(s * SEG, (s + 1) * SEG)
                                em.mm(
                                    ("w", l, k, i2, o2),
                                    zt[:, zsl],
                                    wslice(l, k, i2, o2),
                                    rhs[k // 2, i2][:, sl],
                                    start=False,
                                    stop=last,
                                )
                    # immediate evacuation: o2=0's relu runs under the o2=1
                    # sweep on ACT; o2=1's relu is deferred to the DVE at the
                    # next layer's production point (see pending_z1)
                    if l < NL - 1:
                        nh = hpool.tile([128, T], F16, tag="h", name=f"h_{ti}_{l}_{o2}")
                        if o2 == 0:
                            nc.scalar.activation(nh[:], zt[:], AF.Relu)
                        else:
                            pending_z1 = (zt, nh)
                        newh.append(nh)
                    else:
                        ot = opool.tile([128, T], F16, tag="o", name=f"out_{ti}_{o2}")
                        nc.scalar.activation(ot[:], zt[:], AF.Copy)
                        nc.sync.dma_start(
                            outT[o2 * 128 : (o2 + 1) * 128, t0 : t0 + T], ot[:]
                        )
                    if l == 0 and next_tile is not None:
                        # next tile's gate: logits+exp after the o2=0 sweep
                        # (exp queues ahead of the o2=1 evac on ACT), sT8
                        # matmuls after the o2=1 sweep (eT ready by then)
                        if o2 == 0:
                            load_x(next_tile)
                            gate_st = gate_partA(next_tile)
                        else:
                            gate_partB_mms(gate_st)
                h = newh

        # prologue: tile 0's gate outside the reps loop; each tile's layers
        # then carry the NEXT tile's gate (cyclic across reps)
        load_x(0)
        gate_stage(0)
        ctx = None
        if reps > 1:
            ctx = tc.For_i(0, reps, 1)
            ctx.__enter__()
        for ti in range(NT):
            nxt = ti + 1 if ti + 1 < NT else (0 if reps > 1 else None)
            layers_stage(ti, next_tile=nxt)

        if ctx is not None:
            ctx.__exit__(None, None, None)
        return em


_NC_CACHE = None


def _get_nc():
    global _NC_CACHE
    if _NC_CACHE is None:
        _NC_CACHE = _build_kernel()
    return _NC_CACHE


class _Runner:
    """Persistent sharded PJRT executable for the bass kernel (compile once,
    run many). Mirrors bass2jax.run_bass_via_pjrt's multi-core branch minus
    buffer donation (the kernel writes every output element)."""

    def __init__(self, nc=None):
        import jax
        from jax.sharding import Mesh, PartitionSpec, NamedSharding
        from jax.experimental.shard_map import shard_map
        from concourse import bass2jax, mybir as _mybir

        self.jax = jax
        if nc is None:
            nc = _get_nc()
        bass2jax.install_neuronx_cc_hook()
        part_name = nc.partition_id_tensor.name if nc.partition_id_tensor else None
        in_names, out_names, out_avals, zero_outs = [], [], [], []
        for alloc in nc.m.functions[0].allocations:
            if not isinstance(alloc, _mybir.MemoryLocationSet):
                continue
            name = alloc.memorylocations[0].name
            if alloc.kind == "ExternalInput":
                if name != part_name:
                    in_names.append(name)
            elif alloc.kind == "ExternalOutput":
                out_names.append(name)
                shape = tuple(alloc.tensor_shape)
                dtype = _mybir.dt.np(alloc.dtype)
                out_avals.append(jax.core.ShapedArray(shape, dtype))
                zero_outs.append(np.zeros(shape, dtype))
        self.in_names, self.out_names, self.out_avals = in_names, out_names, out_avals

        bind_names = in_names + out_names + ([part_name] if part_name else [])

        def _body(*args):
            operands = list(args)
            if part_name is not None:
                operands.append(bass2jax.partition_id_tensor())
            outs = bass2jax._bass_exec_p.bind(
                *operands,
                out_avals=tuple(out_avals),
                in_names=tuple(bind_names),
                out_names=tuple(out_names),
                lowering_input_output_aliases=(),
                sim_require_finite=True,
                sim_require_nnan=True,
                nc=nc,
            )
            return tuple(outs)

        devices = jax.devices()[:N_CORES]
        self.mesh = Mesh(np.asarray(devices), ("core",))
        self.spec = PartitionSpec("core")
        self.sharding = NamedSharding(self.mesh, self.spec)
        n_args = len(in_names) + len(out_names)
        self.fn = jax.jit(
            shard_map(
                _body,
                mesh=self.mesh,
                in_specs=(self.spec,) * n_args,
                out_specs=(self.spec,) * len(out_names),
                check_rep=False,
            ),
            keep_unused=True,
        )
        self.zero_outs = [
            jax.device_put(
                np.zeros((N_CORES * z.shape[0], *z.shape[1:]), z.dtype), self.sharding
            )
            for z in zero_outs
        ]

    def device_inputs(self, in_maps):
        concat = [
            np.concatenate([np.asarray(m[name]) for m in in_maps], axis=0)
            for name in self.in_names
        ]
        return [self.jax.device_put(a, self.sharding) for a in concat]

    def run(self, dev_in):
        outs = self.fn(*dev_in, *self.zero_outs)
        return outs

    def to_maps(self, outs):
        res = []
        for c in range(N_CORES):
            res.append(
                {
                    name: np.asarray(outs[i]).reshape(
                        N_CORES, *self.out_avals[i].shape
                    )[c]
                    for i, name in enumerate(self.out_names)
                }
            )
        return res


_RUNNER = None


def _get_runner():
    global _RUNNER
    if _RUNNER is None:
        _RUNNER = _Runner()
    return _RUNNER


def _make_in_maps(x, gate_W, gate_b, block_W, block_b, out_W, out_b):
    x = np.asarray(x, dtype=np.float32)
    xT = np.ascontiguousarray(x.T).astype(np.float16)            # [D, B]
    w_all = np.concatenate(
        [np.asarray(block_W, np.float32), np.asarray(out_W, np.float32)[None]], axis=0
    ).astype(np.float16)                                          # [NL, K, D, D]
    b_all = np.concatenate(
        [np.asarray(block_b, np.float32), np.asarray(out_b, np.float32)[None]], axis=0
    ).astype(np.float16)                                          # [NL, K, D]
    gw = np.asarray(gate_W, np.float32).astype(np.float16)        # [D, K]
    gb = np.asarray(gate_b, np.float32).astype(np.float16).reshape(1, K)
    in_maps = []
    for c in range(N_CORES):
        in_maps.append(
            {
                "xT": np.ascontiguousarray(xT[:, c * BS : (c + 1) * BS]),
                "w": w_all,
                "bb": b_all,
                "gw": gw,
                "gbc": gb.reshape(K, 1),
            }
        )
    return in_maps


def _assemble(results):
    parts = [np.asarray(results[c]["outT"], np.float32).T for c in range(N_CORES)]
    return np.ascontiguousarray(np.concatenate(parts, axis=0))


def kernel(x, gate_W, gate_b, block_W, block_b, out_W, out_b):
    runner = _get_runner()
    in_maps = _make_in_maps(x, gate_W, gate_b, block_W, block_b, out_W, out_b)
    dev_in = runner.device_inputs(in_maps)
    outs = runner.run(dev_in)
    return _assemble(runner.to_maps(outs))


def bench(x, gate_W, gate_b, block_W, block_b, out_W, out_b, iters=20):
    """Returns (output, per_iteration_ns) — steady-state pipelined device time."""
    import time as _time

    runner = _get_runner()
    in_maps = _make_in_maps(x, gate_W, gate_b, block_W, block_b, out_W, out_b)
    dev_in = runner.device_inputs(in_maps)
    outs = runner.run(dev_in)  # warm-up + compile
    for o in outs:
        o.block_until_ready()
    t0 = _time.perf_counter()
    all_outs = [runner.run(dev_in) for _ in range(iters)]
    for outs_i in all_outs:
        for o in outs_i:
            o.block_until_ready()
    t1 = _time.perf_counter()
    per_iter_ns = (t1 - t0) / iters * 1e9
    return _assemble(runner.to_maps(all_outs[-1])), per_iter_ns


# revision 15
# speedup vs baseline: 1.2187x; 1.0786x over previous
"""MoIE (mixture of implicit experts) Trainium2 kernel.

Math (per reference):
    alpha = softmax(x @ gate_W + gate_b)                    # (B, K)
    h = x
    for l in 0..3:  h = relu(sum_k alpha_k * (h @ W[l,k] + b[l,k]))
    out = sum_k alpha_k * (h @ out_W[k] + out_b[k])

Strategy (v2 — o2-major sweeps, split PSUM pools):
  - Data-parallel: shard B=32768 tokens over 8 cores (4096 each); replicate
    the small weights. No collectives.
  - Feature-major on device: activations live as hT [D(part), T(free)] so
    chained matmuls need no activation transposes.
  - alpha folded into the *moving* operand: rhs_k = hT * bcast(alphaT[k]);
    PSUM accumulates over experts and contraction chunks; the per-expert
    bias enters as a tiny alphaT-contraction matmul; the gate bias enters
    as the per-partition bias of the exp() activation.
  - T=1024 tokens per tile: a z accumulator [128,1024]f32 is 2 PSUM banks,
    so zpool holds 3 buffers (6 banks) and the gate gets its own 2-bank
    pool — the pipelined gate no longer serializes the next layer's PSUM
    allocation (the 427us build shared one 2-buffer pool for both).
  - o2-major layer sweeps: all (k,i2) expert matmuls for output-half 0
    accumulate + evacuate BEFORE the o2=1 sweep runs. z[o2=0] therefore
    finishes mid-layer and its relu/evac + the next layer's DVE rhs
    production hide under ~7us of o2=1 matmuls (the 427us build finished
    both halves at the layer end, exposing the evac->rhs chain).
  - rhs tiles live across both o2 sweeps: rpool bufs=24 (2KB/partition
    each) so the DVE runs a full layer ahead.
  - fp16 matmul path, fp32 PSUM; fp16 output bounce.
  - LDWEIGHTS dedupe within same-stationary groups (PE stream pinned to
    program order); measured 1055/1062 LDW hidden under matmuls.
  - Software-pipelined gating: tile ti's layer pass carries tile ti+1's
    gate (cyclic across reps at the last tile); with NT=4 the next-rep
    broadcast lands ~3 layer-tiles before the For_i barrier.
"""

import sys

if "/opt/trn_rl_repo" not in sys.path:
    sys.path.insert(0, "/opt/trn_rl_repo")

import numpy as np

import concourse.bass as bass
import concourse.bass_isa as bass_isa
import concourse.tile as tile
import concourse.mybir as mybir
from concourse import bacc
from concourse.bass import _add_dep_helper
from concourse.bass_utils import run_bass_kernel_spmd

N_CORES = 8
B, D, K, L = 32768, 256, 8, 4
NL = L + 1                  # 4 hidden blocks + output block
BS = B // N_CORES           # 4096 tokens per core
T = 1024                    # tokens per on-chip tile
NT = BS // T                # tiles per core
SEG = 512                   # f32 PSUM bank = 512 elements
NSEG = T // SEG
F16 = mybir.dt.float16
F32 = mybir.dt.float32
AF = mybir.ActivationFunctionType
_RPOOL_BUFS = 18
_APOOL_BUFS = 2
_ZPOOL_BUFS = 3
_GPOOL_BUFS = 2
_LDW_DEDUP = True


class _MMEmitter:
    """Emit matmuls, tracking which ones share a stationary operand with the
    immediately preceding matmul. Tile's legalizer splits every InstMatmult
    into InstLdweights + InstMatmult; `_dedupe_ldweights` later deletes the
    redundant loads for the marked matmuls. A nosync dep chain pins the PE
    stream to program order so a dedup'd matmul can never observe a foreign
    group's weights."""

    def __init__(self, nc):
        self.nc = nc
        self.key = None
        self.prev = None
        self.skip_names = set()

    def mm(self, key, out, lhsT, rhs, start, stop, tp=None):
        bi = self.nc.tensor.matmul(
            out, lhsT, rhs, start=start, stop=stop, tile_position=tp
        )
        if _LDW_DEDUP:
            if self.prev is not None:
                _add_dep_helper(
                    bi.ins, self.prev, sync=False, reason="pe-program-order"
                )
            if key is not None and key == self.key:
                self.skip_names.add(bi.ins.name)
        self.key = key
        self.prev = bi.ins
        return bi


def _dedupe_ldweights(nc, skip_names):
    """Remove the InstLdweights preceding each marked matmul (same stationary
    as the previous matmul, PE stream pinned to program order). Waits are
    moved onto the matmul; dependency edges are merged / remapped."""
    removed = {}
    for b in nc.m.functions[0].blocks:
        insts = list(b.instructions)
        keep = [True] * len(insts)
        for idx, ins in enumerate(insts):
            if not (isinstance(ins, mybir.InstMatmult) and ins.name in skip_names):
                continue
            j = idx - 1
            lw = None
            while j >= 0:
                pj = insts[j]
                if isinstance(pj, mybir.InstLdweights):
                    if keep[j]:
                        lw = pj
                    break
                if isinstance(pj, mybir.InstMatmult):
                    break
                j -= 1
            if lw is None:
                continue
            si = lw.sync_info
            if si is not None and len(si.on_update) > 0:
                continue  # LDW signals a semaphore: leave it alone
            if si is not None and len(si.on_wait) > 0:
                msi = ins.sync_info
                waits = list(si.on_wait) + (
                    list(msi.on_wait) if msi is not None else []
                )
                upds = list(msi.on_update) if msi is not None else []
                ins.sync_info = mybir.SyncInfo(on_wait=waits, on_update=upds)
            ins.merge_dependencies_from(lw)
            keep[j] = False
            removed[lw.name] = ins.name
        if not all(keep):
            b.instructions = [i for i, k in zip(insts, keep) if k]
    if removed:
        for b in nc.m.functions[0].blocks:
            for i in b.instructions:
                i.remap_dependency_names(removed)
    return len(removed)


def _build_kernel(reps=1):
    nc = bacc.Bacc(
        "TRN2",
        target_bir_lowering=False,
        debug=False,
        enable_asserts=False,
        num_devices=N_CORES,
    )
    xT = nc.dram_tensor("xT", [D, BS], F16, kind="ExternalInput").ap()
    w = nc.dram_tensor("w", [NL, K, D, D], F16, kind="ExternalInput").ap()
    bb = nc.dram_tensor("bb", [NL, K, D], F16, kind="ExternalInput").ap()
    gw = nc.dram_tensor("gw", [D, K], F16, kind="ExternalInput").ap()
    gbc = nc.dram_tensor("gbc", [K, 1], F16, kind="ExternalInput").ap()
    outT = nc.dram_tensor("outT", [D, BS], F16, kind="ExternalOutput").ap()

    with tile.TileContext(nc) as tc:
        em = _body(nc, tc, xT, w, bb, gw, gbc, outT, reps)
    if _LDW_DEDUP and em is not None:
        n = _dedupe_ldweights(nc, em.skip_names)
        assert n == len(em.skip_names), (n, len(em.skip_names))
    nc.compile()
    return nc


def _body(nc, tc, xT, w, bb, gw, gbc_d, outT, reps=1):
    with (
        tc.tile_pool(name="cpool", bufs=1) as cpool,
        tc.tile_pool(name="xpool", bufs=2 * NT) as xpool,
        tc.tile_pool(name="hpool", bufs=6) as hpool,
        tc.tile_pool(name="rpool", bufs=_RPOOL_BUFS) as rpool,
        tc.tile_pool(name="apool", bufs=_APOOL_BUFS) as apool,
        tc.tile_pool(name="spool", bufs=2) as spool,
        tc.tile_pool(name="opool", bufs=2) as opool,
        tc.tile_pool(name="dpool", bufs=2, space=bass.MemorySpace.DRAM) as dpool,
        tc.tile_pool(name="zpool", bufs=_ZPOOL_BUFS, space=bass.MemorySpace.PSUM) as zpool,
        tc.tile_pool(name="gpool", bufs=_GPOOL_BUFS, space=bass.MemorySpace.PSUM) as gpool,
    ):
        # ---- small constants first (the HWDGE queue is FIFO: keep the
        # gate/bias/x transfers ahead of the 5MB weight stream) ----
        gwt = cpool.tile([128, 2 * K], F16, name="gwt")
        for i2 in range(2):
            nc.sync.dma_start(
                gwt[:, i2 * K : (i2 + 1) * K], gw[i2 * 128 : (i2 + 1) * 128, :]
            )
        gbc = cpool.tile([K, 1], F16, name="gbc")
        nc.sync.dma_start(gbc[:], gbc_d[:])
        bt = cpool.tile([K, NL * D], F16, name="bt")
        # second bias copy on partitions 32-39: feeds the o2=1 bias matmul at
        # array rows 32-63 (tile_position=(32,0)) so both o2 bias matmuls run
        # concurrently in separate row groups
        bt40 = cpool.tile([40, NL * D], F16, name="bt40")
        ones8x8 = cpool.tile([K, K], F16, name="ones8x8")
        nc.vector.memset(ones8x8[:], 1.0)
        wt = cpool.tile([128, NL * K * 2 * D], F16, name="wt")

        def load_weights():
            for l in range(NL):
                nc.sync.dma_start(bt[:, l * D : (l + 1) * D], bb[l])
                nc.sync.dma_start(bt40[32:40, l * D : (l + 1) * D], bb[l])
            for l in range(NL):
                for k in range(K):
                    for i2 in range(2):
                        off = ((l * K + k) * 2 + i2) * D
                        nc.sync.dma_start(
                            wt[:, off : off + D],
                            w[l, k, i2 * 128 : (i2 + 1) * 128, :],
                        )

        def wslice(l, k, i2, o2):
            base = ((l * K + k) * 2 + i2) * D + o2 * 128
            return wt[:, base : base + 128]

        em = _MMEmitter(nc)

        if reps > 1:
            # steady-state benchmarking variant: weights resident across reps
            load_weights()

        hs, alphaTs, a40s, abcs = {}, {}, {}, {}
        emit_seq = [0]

        def load_x(ti):
            sq = emit_seq[0]
            t0 = ti * T
            h = []
            for i2 in range(2):
                ht = xpool.tile([128, T], F16, tag="x", name=f"x_{ti}_{i2}_{sq}")
                for s in range(NSEG):
                    sl = slice(s * SEG, (s + 1) * SEG)
                    nc.sync.dma_start(
                        ht[:, sl],
                        xT[i2 * 128 : (i2 + 1) * 128, t0 + s * SEG : t0 + (s + 1) * SEG],
                    )
                h.append(ht)
            hs[ti] = h

        def gate_partA(ti):
            """Gate logits glT[k,t] = gate_W.T @ x per half-tile + exp().
            Emitted right after L0's o2=0 sweep so the exp ACT instruction
            queues ahead of the o2=1 evacuation (the sT8 matmuls, emitted
            after the o2=1 sweep, then find eT ready)."""
            sq = emit_seq[0]
            emit_seq[0] += 1
            h = hs[ti]
            eT = spool.tile([K, T], F16, tag="eT", name=f"eT_{ti}_{sq}", bufs=1)
            gate_stats = [
                (gwt[:, 0:K], h[0]),
                (gwt[:, K : 2 * K], h[1]),
            ]
            nstat = len(gate_stats)
            HT = T // 2
            glTs = []
            for hf in range(2):
                glT = gpool.tile([K, HT], F32, tag="g", name=f"glT_{ti}_{hf}_{sq}")
                for wi, (stat, mov) in enumerate(gate_stats):
                    for s in range(NSEG // 2):
                        sl = slice(s * SEG, (s + 1) * SEG)
                        gsl = slice(hf * HT + s * SEG, hf * HT + (s + 1) * SEG)
                        em.mm(
                            ("gate", wi),
                            glT[:, sl],
                            stat,
                            mov[:, gsl],
                            start=(wi == 0),
                            stop=(wi == nstat - 1),
                        )
                # softmax over the 8 partitions (no max-subtract needed;
                # logits are ~N(0,1) so exp() is safe); gate bias enters as
                # the ACT per-partition bias vector
                hsl = slice(hf * HT, (hf + 1) * HT)
                nc.scalar.activation(eT[:, hsl], glT[:], AF.Exp, bias=gbc[:])
                glTs.append(glT)
            return {"ti": ti, "sq": sq, "eT": eT}

        def gate_partB_mms(st):
            """sum-over-experts matmuls: all-ones [8,8] lhsT -> every row is
            sum_k e_k. Emitted after L0's o2=1 sweep (eT is ready by then)."""
            ti, sq, eT = st["ti"], st["sq"], st["eT"]
            HT = T // 2
            sT8s = []
            for hf in range(2):
                sT8 = gpool.tile([K, HT], F32, tag="g", name=f"sT8_{ti}_{hf}_{sq}")
                for s in range(NSEG // 2):
                    sl = slice(s * SEG, (s + 1) * SEG)
                    esl = slice(hf * HT + s * SEG, hf * HT + (s + 1) * SEG)
                    em.mm(
                        ("ones8",),
                        sT8[:, sl],
                        ones8x8[:],
                        eT[:, esl],
                        start=True,
                        stop=True,
                    )
                sT8s.append(sT8)
            st["sT8s"] = sT8s

        def gate_partB_rest(st):
            """reciprocal + alpha = e/sum + DRAM-bounce broadcast. Emitted
            after L1's rhs production so the DVE FIFO reaches the next
            layer's rhs muls before blocking on the sT8 result."""
            ti, sq, eT = st["ti"], st["sq"], st["eT"]
            HT = T // 2
            r8 = spool.tile([K, T], F16, tag="rT", name=f"r8_{ti}_{sq}", bufs=1)
            l8 = spool.tile([K, T], F16, tag="l8", name=f"l8_{ti}_{sq}", bufs=1)
            # 1/S division-free on ACT: exp(-ln S). The DVE's iterative-divide
            # RECIPROCAL costs ~4us per half-tile and stalled the rhs pipeline;
            # the Ln/Exp LUT error (~1e-3) is far inside the 2e-2 budget.
            with nc.allow_low_precision("fp16 softmax normalizer"):
                for hf in range(2):
                    hsl = slice(hf * HT, (hf + 1) * HT)
                    nc.scalar.activation(l8[:, hsl], st["sT8s"][hf][:], AF.Ln)
                    nc.scalar.activation(r8[:, hsl], l8[:, hsl], AF.Exp, scale=-1.0)
            alphaT = spool.tile([K, T], F16, tag="alphaT", name=f"alphaT_{ti}_{sq}")
            nc.vector.tensor_mul(alphaT[:], eT[:], r8[:])

            # broadcast alphaT rows across all 128 partitions so the DVE can
            # multiply h by alpha_k elementwise: bounce through DRAM (SBUF-
            # source broadcast APs are unsupported), then step-0 DRAM->SBUF
            # broadcast DMAs on the otherwise-idle GpSimd queue + Sync
            abc = apool.tile([128, K * T], F16, tag="abc", name=f"abc_{ti}_{sq}")
            adram = dpool.tile([K, T], F16, tag="adram", name=f"adram_{ti}_{sq}")
            nc.gpsimd.dma_start(adram[:], alphaT[:])
            bengs = [nc.gpsimd, nc.sync]
            for k in range(K):
                bengs[k % len(bengs)].dma_start(
                    abc[:, k * T : (k + 1) * T],
                    adram[k : k + 1, :].broadcast_to([128, T]),
                )
            # alphaT copy on partitions 32-39 for the row-packed o2=1 bias
            a40 = spool.tile([40, T], F16, tag="a32", name=f"a40_{ti}_{sq}")
            nc.gpsimd.dma_start(a40[32:40, :], alphaT[:])
            alphaTs[ti] = alphaT
            a40s[ti] = a40
            abcs[ti] = abc

        def gate_stage(ti):
            """Standalone gate (prologue only)."""
            st = gate_partA(ti)
            gate_partB_mms(st)
            gate_partB_rest(st)
            if ti == 0 and reps == 1:
                load_weights()

        def layers_stage(ti, next_tile=None):
            t0 = ti * T
            h = hs[ti]
            alphaT = alphaTs[ti]
            a40 = a40s[ti]
            abc = abcs[ti]
            gate_st = None
            pending_z1 = None
            for l in range(NL):
                # rhs production i2-major (the i2=0 tiles depend only on
                # h[0], evacuated mid-previous-layer) and pair-batched: one
                # DVE op covers 2 experts (abc is expert-contiguous; h enters
                # via a stride-0 repeat dim), amortizing the DVE's 151-cycle
                # per-op overhead: 9.8us/layer instead of 11us.
                rhs = {}
                for i2 in range(2):
                    if i2 == 1 and pending_z1 is not None:
                        # previous layer's o2=1 relu runs on the DVE itself,
                        # placed right before its consumers: no cross-engine
                        # hop on the z1-stop -> h[1] -> rhs critical chain,
                        # and the i2=0 production above stays unblocked
                        zprev, nhprev = pending_z1
                        for s in range(NSEG):
                            ssl = slice(s * SEG, (s + 1) * SEG)
                            nc.vector.tensor_relu(nhprev[:, ssl], zprev[:, ssl])
                        pending_z1 = None
                    for kp in range(K // 2):
                        rt = rpool.tile(
                            [128, 2 * T], F16, tag="rhs", name=f"rhs_{ti}_{l}_{kp}_{i2}"
                        )
                        nc.vector.tensor_mul(
                            rt[:].rearrange("p (j t) -> p j t", j=2),
                            h[i2][:].unsqueeze(1).broadcast_to([128, 2, T]),
                            abc[:, 2 * kp * T : (2 * kp + 2) * T].rearrange(
                                "p (j t) -> p j t", j=2
                            ),
                        )
                        rhs[kp, i2] = rt
                if l == 1 and gate_st is not None:
                    # gate part B tail sits after this layer's rhs muls in
                    # the DVE FIFO (the reciprocal waits on sT8; anything
                    # emitted after it would stall behind that wait)
                    gate_partB_rest(gate_st)

                zts = [
                    zpool.tile([128, T], F32, tag="z", name=f"z_{ti}_{l}_{o2}")
                    for o2 in range(2)
                ]
                # both bias matmuls up front: o2=0 in array rows 0-31, o2=1 in
                # rows 32-63 (tile_position) — they run concurrently, halving
                # the bias stream time; each starts its z accumulation group
                for s in range(NSEG):
                    sl = slice(s * SEG, (s + 1) * SEG)
                    em.mm(
                        ("bt", l, 0),
                        zts[0][:, sl],
                        bt[:, l * D : l * D + 128],
                        alphaT[:, sl],
                        start=True,
                        stop=False,
                    )
                for s in range(NSEG):
                    sl = slice(s * SEG, (s + 1) * SEG)
                    em.mm(
                        ("bt", l, 1),
                        zts[1][:, sl],
                        bt40[32:40, l * D + 128 : l * D + 256],
                        a40[32:40, sl],
                        start=True,
                        stop=False,
                        tp=(32, 0),
                    )
                newh = []
                for o2 in range(2):
                    zt = zts[o2]
                    for i2 in range(2):
                        for k in range(K):
                            last = (k == K - 1) and (i2 == 1)
                            for s in range(NSEG):
                                sl = slice((k % 2) * T + s * SEG, (k % 2) * T + (s + 1) * SEG)
                                zsl = slice(s * SEG, (s + 1) * SEG)
                                em.mm(
                                    ("w", l, k, i2, o2),
                                    zt[:, zsl],
                                    wslice(l, k, i2, o2),
                                    rhs[k // 2, i2][:, sl],
                                    start=False,
                                    stop=last,
                                )
                    # immediate evacuation: o2=0's relu runs under the o2=1
                    # sweep on ACT; o2=1's relu is deferred to the DVE at the
                    # next layer's production point (see pending_z1)
                    if l < NL - 1:
                        nh = hpool.tile([128, T], F16, tag="h", name=f"h_{ti}_{l}_{o2}")
                        if o2 == 0:
                            nc.scalar.activation(nh[:], zt[:], AF.Relu)
                        else:
                            pending_z1 = (zt, nh)
                        newh.append(nh)
                    else:
                        ot = opool.tile([128, T], F16, tag="o", name=f"out_{ti}_{o2}")
                        nc.scalar.activation(ot[:], zt[:], AF.Copy)
                        nc.sync.dma_start(
                            outT[o2 * 128 : (o2 + 1) * 128, t0 : t0 + T], ot[:]
                        )
                    if l == 0 and next_tile is not None:
                        # next tile's gate: logits+exp after the o2=0 sweep
                        # (exp queues ahead of the o2=1 evac on ACT), sT8
                        # matmuls after the o2=1 sweep (eT ready by then)
                        if o2 == 0:
                            load_x(next_tile)
                            gate_st = gate_partA(next_tile)
                        else:
                            gate_partB_mms(gate_st)
                h = newh

        # prologue: tile 0's gate outside the reps loop; each tile's layers
        # then carry the NEXT tile's gate (cyclic across reps)
        load_x(0)
        gate_stage(0)
        ctx = None
        if reps > 1:
            ctx = tc.For_i(0, reps, 1)
            ctx.__enter__()
        for ti in range(NT):
            nxt = ti + 1 if ti + 1 < NT else (0 if reps > 1 else None)
            layers_stage(ti, next_tile=nxt)

        if ctx is not None:
            ctx.__exit__(None, None, None)
        return em


_NC_CACHE = None


def _get_nc():
    global _NC_CACHE
    if _NC_CACHE is None:
        _NC_CACHE = _build_kernel()
    return _NC_CACHE


class _Runner:
    """Persistent sharded PJRT executable for the bass kernel (compile once,
    run many). Mirrors bass2jax.run_bass_via_pjrt's multi-core branch minus
    buffer donation (the kernel writes every output element)."""

    def __init__(self, nc=None):
        import jax
        from jax.sharding import Mesh, PartitionSpec, NamedSharding
        from jax.experimental.shard_map import shard_map
        from concourse import bass2jax, mybir as _mybir

        self.jax = jax
        if nc is None:
            nc = _get_nc()
        bass2jax.install_neuronx_cc_hook()
        part_name = nc.partition_id_tensor.name if nc.partition_id_tensor else None
        in_names, out_names, out_avals, zero_outs = [], [], [], []
        for alloc in nc.m.functions[0].allocations:
            if not isinstance(alloc, _mybir.MemoryLocationSet):
                continue
            name = alloc.memorylocations[0].name
            if alloc.kind == "ExternalInput":
                if name != part_name:
                    in_names.append(name)
            elif alloc.kind == "ExternalOutput":
                out_names.append(name)
                shape = tuple(alloc.tensor_shape)
                dtype = _mybir.dt.np(alloc.dtype)
                out_avals.append(jax.core.ShapedArray(shape, dtype))
                zero_outs.append(np.zeros(shape, dtype))
        self.in_names, self.out_names, self.out_avals = in_names, out_names, out_avals

        bind_names = in_names + out_names + ([part_name] if part_name else [])

        def _body(*args):
            operands = list(args)
            if part_name is not None:
                operands.append(bass2jax.partition_id_tensor())
            outs = bass2jax._bass_exec_p.bind(
                *operands,
                out_avals=tuple(out_avals),
                in_names=tuple(bind_names),
                out_names=tuple(out_names),
                lowering_input_output_aliases=(),
                sim_require_finite=True,
                sim_require_nnan=True,
                nc=nc,
            )
            return tuple(outs)

        devices = jax.devices()[:N_CORES]
        self.mesh = Mesh(np.asarray(devices), ("core",))
        self.spec = PartitionSpec("core")
        self.sharding = NamedSharding(self.mesh, self.spec)
        n_args = len(in_names) + len(out_names)
        self.fn = jax.jit(
            shard_map(
                _body,
                mesh=self.mesh,
                in_specs=(self.spec,) * n_args,
                out_specs=(self.spec,) * len(out_names),
                check_rep=False,
            ),
            keep_unused=True,
        )
        self.zero_outs = [
            jax.device_put(
                np.zeros((N_CORES * z.shape[0], *z.shape[1:]), z.dtype), self.sharding
            )
            for z in zero_outs
        ]

    def device_inputs(self, in_maps):
        concat = [
            np.concatenate([np.asarray(m[name]) for m in in_maps], axis=0)
            for name in self.in_names
        ]
        return [self.jax.device_put(a, self.sharding) for a in concat]

    def run(self, dev_in):
        outs = self.fn(*dev_in, *self.zero_outs)
        return outs

    def to_maps(self, outs):
        res = []
        for c in range(N_CORES):
            res.append(
                {
                    name: np.asarray(outs[i]).reshape(
                        N_CORES, *self.out_avals[i].shape
                    )[c]
                    for i, name in enumerate(self.out_names)
                }
            )
        return res


_RUNNER = None


def _get_runner():
    global _RUNNER
    if _RUNNER is None:
        _RUNNER = _Runner()
    return _RUNNER


def _make_in_maps(x, gate_W, gate_b, block_W, block_b, out_W, out_b):
    x = np.asarray(x, dtype=np.float32)
    xT = np.ascontiguousarray(x.T).astype(np.float16)            # [D, B]
    w_all = np.concatenate(
        [np.asarray(block_W, np.float32), np.asarray(out_W, np.float32)[None]], axis=0
    ).astype(np.float16)                                          # [NL, K, D, D]
    b_all = np.concatenate(
        [np.asarray(block_b, np.float32), np.asarray(out_b, np.float32)[None]], axis=0
    ).astype(np.float16)                                          # [NL, K, D]
    gw = np.asarray(gate_W, np.float32).astype(np.float16)        # [D, K]
    gb = np.asarray(gate_b, np.float32).astype(np.float16).reshape(1, K)
    in_maps = []
    for c in range(N_CORES):
        in_maps.append(
            {
                "xT": np.ascontiguousarray(xT[:, c * BS : (c + 1) * BS]),
                "w": w_all,
                "bb": b_all,
                "gw": gw,
                "gbc": gb.reshape(K, 1),
            }
        )
    return in_maps


def _assemble(results):
    parts = [np.asarray(results[c]["outT"], np.float32).T for c in range(N_CORES)]
    return np.ascontiguousarray(np.concatenate(parts, axis=0))


def kernel(x, gate_W, gate_b, block_W, block_b, out_W, out_b):
    runner = _get_runner()
    in_maps = _make_in_maps(x, gate_W, gate_b, block_W, block_b, out_W, out_b)
    dev_in = runner.device_inputs(in_maps)
    outs = runner.run(dev_in)
    return _assemble(runner.to_maps(outs))


def bench(x, gate_W, gate_b, block_W, block_b, out_W, out_b, iters=20):
    """Returns (output, per_iteration_ns) — steady-state pipelined device time."""
    import time as _time

    runner = _get_runner()
    in_maps = _make_in_maps(x, gate_W, gate_b, block_W, block_b, out_W, out_b)
    dev_in = runner.device_inputs(in_maps)
    outs = runner.run(dev_in)  # warm-up + compile
    for o in outs:
        o.block_until_ready()
    t0 = _time.perf_counter()
    all_outs = [runner.run(dev_in) for _ in range(iters)]
    for outs_i in all_outs:
        for o in outs_i:
            o.block_until_ready()
    t1 = _time.perf_counter()
    per_iter_ns = (t1 - t0) / iters * 1e9
    return _assemble(runner.to_maps(all_outs[-1])), per_iter_ns
